# revision 1
# baseline (speedup 1.0000x reference)
"""AECF multimodal fusion kernel for 8 TRN2 NeuronCores.

Strategy:
  - Host-side routing (part of sharding): rows are sorted into three branch
    groups (both modalities present / only-image / only-text) using the same
    norm>1e-6 predicate as the reference. Each group is dealt evenly across
    the 8 cores and padded to a tile multiple; the NEFF is compiled with the
    actual per-core group sizes (compile happens inside kernel(), after the
    inputs are known), so the graph is static and identical on all cores.
  - Feature-major layout everywhere: inputs are shipped as x^T [512, n]
    (host pre-transpose), all matmuls keep the contraction dim on SBUF
    partitions, and the output leaves the device as logits^T [80, n].
  - bf16 storage/compute, f32 PSUM accumulation.
  - Algebra: scores only enter through a 2-way softmax, so
    attn_img = sigmoid((s_img - s_txt)) with s_s = enc_s @ Wk_eff,
    Wk_eff[:, h] = Wk[:, 64h:64h+64] @ q[h] / 8 (the bk term cancels in the
    difference).  Wo and W_fp are merged into Wof = Wo @ W_fp.  The pooled
    value uses pooled = v_txt + A*(v_img - v_txt) where A is attn_img
    broadcast over each 64-wide head block (one small PE matmul per chunk).
"""

import os
import sys

if "/opt/trn_rl_repo" not in sys.path:
    sys.path.insert(0, "/opt/trn_rl_repo")

import numpy as np
import ml_dtypes

import concourse.bass as bass
import concourse.bacc as bacc
import concourse.tile as tile
from concourse import mybir
from concourse.bass_utils import run_bass_kernel_spmd

BF = mybir.dt.bfloat16
F32 = mybir.dt.float32
AF = mybir.ActivationFunctionType
OP = mybir.AluOpType

H = 256
ID = 512
TD = 512
NCLS = 80
NH = 4
HD = 64
B = 131072
NCORES = 8
T = 512  # batch-tile (free-dim) size
TM = 512  # matmul max free dim (one psum bank of f32)

LAST_EXEC_NS = None
LAST_PROFILE = None

_GRAPH_CACHE = {}


def _build_graph(nb, ni, nt, zero_bias):
    """Build the SPMD graph for per-core group column counts nb/ni/nt
    (each a multiple of T, possibly 0). zero_bias selects an evacuation
    scheme that keeps every ACT function inside one table set (Copy/Relu/
    Sigmoid all live in sigmoid_and_others, so no ACT_TABLE_LOAD thrash)."""
    nc = bacc.Bacc()

    # ---- DRAM I/O ----
    dram = {}
    if nb:
        dram["xb_img"] = nc.dram_tensor("xb_img", [ID, nb], BF, kind="ExternalInput")
        dram["xb_txt"] = nc.dram_tensor("xb_txt", [TD, nb], BF, kind="ExternalInput")
        dram["outb"] = nc.dram_tensor("outb", [NCLS, nb], F32, kind="ExternalOutput")
    if ni:
        dram["xi_img"] = nc.dram_tensor("xi_img", [ID, ni], BF, kind="ExternalInput")
        dram["outi"] = nc.dram_tensor("outi", [NCLS, ni], F32, kind="ExternalOutput")
    if nt:
        dram["xt_txt"] = nc.dram_tensor("xt_txt", [TD, nt], BF, kind="ExternalInput")
        dram["outt"] = nc.dram_tensor("outt", [NCLS, nt], F32, kind="ExternalOutput")

    # weights (feature-major, pre-tiled on host)
    wspec = {
        "wie": ([128, 4, H], BF),
        "wte": ([128, 4, H], BF),
        "wv": ([128, 2, H], BF),
        "wkeff": ([128, 2, NH], BF),
        "emat": ([NH, 2, 128], BF),
        "wof": ([128, 2, 2 * H], BF),
        "wip": ([128, 2, 2 * H], BF),
        "wtp": ([128, 2, 2 * H], BF),
        "wc1": ([128, 4, H], BF),
        "wc2": ([128, 2, NCLS], BF),
        "bie": ([128, 2], F32),
        "bte": ([128, 2], F32),
        "bv": ([128, 2], F32),
        "bof": ([128, 4], F32),
        "bip": ([128, 4], F32),
        "btp": ([128, 4], F32),
        "bc1": ([128, 2], F32),
        "bc2": ([128, 1], F32),
    }
    for name, (shape, dt) in wspec.items():
        dram[name] = nc.dram_tensor(name, shape, dt, kind="ExternalInput")

    NHALF = T // TM  # matmuls per psum tile (psum bank = 512 f32)

    with tile.TileContext(nc) as tc:
        with (
            tc.tile_pool(name="wpool", bufs=1) as wpool,
            tc.tile_pool(name="work", bufs=4) as wp,
            tc.tile_pool(name="psum", bufs=8, space="PSUM") as pp,
        ):
            w = {}
            for name, (shape, dt) in wspec.items():
                w[name] = wpool.tile(shape, dt, tag=name, name=name)
                nc.gpsimd.dma_start(w[name][:], dram[name][:])


            def evac(dst_ap, src_ap, bias_ap, engine):
                """Copy psum->sbuf with optional per-partition bias add."""
                if zero_bias:
                    if engine == "act":
                        nc.scalar.activation(dst_ap, src_ap, AF.Copy)
                    else:
                        nc.vector.tensor_copy(dst_ap, src_ap)
                else:
                    if engine == "act":
                        nc.scalar.activation(dst_ap, src_ap, AF.Identity, bias=bias_ap)
                    else:
                        nc.vector.tensor_scalar_add(dst_ap, src_ap, bias_ap)

            def mm_group(ps_ap, lhsT, rhs, ks, pdim=128):
                """ps_ap [pdim, T] accumulated over ks K-tiles of rhs
                [128, nk, T], split into TM-wide column halves."""
                for hh in range(NHALF):
                    for k in range(ks):
                        nc.tensor.matmul(
                            ps_ap[:pdim, hh * TM : (hh + 1) * TM],
                            lhsT(k),
                            rhs[:, k, hh * TM : (hh + 1) * TM],
                            start=(k == 0),
                            stop=(k == ks - 1),
                        )

            def mlp_head(fused_sb, out_ap, j):
                """fused_sb [128,4,T] bf16 -> relu(@Wc1+bc1) -> @Wc2+bc2 ->
                DMA to out_ap[:, j*T:(j+1)*T]"""
                h1 = wp.tile([128, 2, T], BF, tag="h1", name="h1")
                for m in range(2):
                    ps = pp.tile([128, T], F32, tag="ps", name="ps")
                    mm_group(ps[:], lambda k: w["wc1"][:, k, m * 128 : (m + 1) * 128],
                             fused_sb, 4)
                    nc.scalar.activation(
                        h1[:, m, :], ps[:], AF.Relu, bias=w["bc1"][:, m : m + 1]
                    )
                ps = pp.tile([128, T], F32, tag="ps", name="ps")
                mm_group(ps[:], lambda k: w["wc2"][:, k, :], h1, 2, pdim=NCLS)
                osb = wp.tile([NCLS, T], F32, tag="osb", name="osb")
                evac(osb[:], ps[:NCLS, :], w["bc2"][:NCLS, :], "act")
                nc.gpsimd.dma_start(out_ap[:, j * T : (j + 1) * T], osb[:])

            def encoder(x_ap, j, wname, bname, tag):
                tag = "i" if tag in ("bi", "oi") else "t"
                """DMA input tile + 2-chunk encoder with relu. Returns
                enc [128,2,T] bf16."""
                xi = wp.tile([128, 4, T], BF, tag="x" + tag, name="x" + tag)
                eng = nc.sync if tag == "i" else nc.scalar
                eng.dma_start(xi[:], x_ap[:, :, j * T : (j + 1) * T])
                enc = wp.tile([128, 2, T], BF, tag="enc" + tag, name="enc" + tag)
                for m in range(2):
                    ps = pp.tile([128, T], F32, tag="ps", name="ps")
                    mm_group(ps[:], lambda k: w[wname][:, k, m * 128 : (m + 1) * 128],
                             xi, 4)
                    nc.scalar.activation(
                        enc[:, m, :], ps[:], AF.Relu, bias=w[bname][:, m : m + 1]
                    )
                return enc

            def proj_512(dst_sb, src_sb, wname, bname, engine):
                """dst[128,4,T] = src[128,2,T] @ w[wname] + b; evacuation on
                the given engine ('act' or 'dve')."""
                for m in range(4):
                    ps = pp.tile([128, T], F32, tag="ps", name="ps")
                    mm_group(ps[:], lambda k: w[wname][:, k, m * 128 : (m + 1) * 128],
                             src_sb, 2)
                    eng = engine if engine in ("act", "dve") else ("act" if m < 2 else "dve")
                    evac(dst_sb[:, m, :], ps[:], w[bname][:, m : m + 1], eng)

            def only_pipe(x_ap, out_ap, ntiles, wenc, benc, wproj, bproj, tag):
                pend = {0: encoder(x_ap, 0, wenc, benc, tag)}
                for j in range(ntiles):
                    enc = pend.pop(j)
                    fused = wp.tile([128, 4, T], BF, tag="fused", name="fused")
                    proj_512(fused, enc, wproj, bproj, "mix")
                    if j + 1 < ntiles:
                        pend[j + 1] = encoder(x_ap, j + 1, wenc, benc, tag)
                    mlp_head(fused, out_ap, j)

            # ---------- both-modality pipeline ----------
            # Software-pipelined emission: tile j+1's input DMA + encoder
            # matmuls are emitted between tile j's attention phase and its
            # pooled-dependent matmuls, so the in-order PE stream has work
            # while ACT/DVE produce attn/pooled for tile j.
            if nb:
                xbi_ap = dram["xb_img"][:].rearrange("(k p) n -> p k n", p=128)
                xbt_ap = dram["xb_txt"][:].rearrange("(k p) n -> p k n", p=128)
                nbt = nb // T
                pend = {0: (encoder(xbi_ap, 0, "wie", "bie", "bi"),
                            encoder(xbt_ap, 0, "wte", "bte", "bt"))}
                for j in range(nbt):
                    enc_i, enc_t = pend.pop(j)

                    # d_enc = enc_i - enc_t  (for the score difference)
                    denc = wp.tile([128, 2, T], BF, tag="denc", name="denc")
                    nc.vector.tensor_tensor(denc[:], enc_i[:], enc_t[:], op=OP.subtract)

                    # delta scores [4, T] and attn = sigmoid(delta)
                    psd = pp.tile([128, T], F32, tag="ps", name="ps")
                    mm_group(psd[:], lambda k: w["wkeff"][:, k, :], denc, 2, pdim=NH)
                    attn = wp.tile([NH, T], BF, tag="attn", name="attn")
                    nc.scalar.activation(attn[:], psd[:NH, :], AF.Sigmoid)

                    # v projections for both slots
                    ps_vi = []
                    vt = wp.tile([128, 2, T], BF, tag="vt", name="vt")
                    for m in range(2):
                        pv = pp.tile([128, T], F32, tag="ps", name="ps")
                        mm_group(pv[:], lambda k: w["wv"][:, k, m * 128 : (m + 1) * 128],
                                 enc_i, 2)
                        ps_vi.append(pv)
                    for m in range(2):
                        pv = pp.tile([128, T], F32, tag="ps", name="ps")
                        mm_group(pv[:], lambda k: w["wv"][:, k, m * 128 : (m + 1) * 128],
                                 enc_t, 2)
                        evac(vt[:, m, :], pv[:], w["bv"][:, m : m + 1], "act")

                    # A = head-broadcast(attn) on PE
                    ps_a = []
                    for m in range(2):
                        pa = pp.tile([128, T], F32, tag="ps", name="ps")
                        for hh in range(NHALF):
                            nc.tensor.matmul(
                                pa[:, hh * TM : (hh + 1) * TM],
                                w["emat"][:, m, :],
                                attn[:, hh * TM : (hh + 1) * TM],
                                start=True,
                                stop=True,
                            )
                        ps_a.append(pa)

                    # hoist next tile's input DMA + encoder matmuls here:
                    # they fill the PE while DVE computes dd/tmp/pooled below.
                    if j + 1 < nbt:
                        pend[j + 1] = (encoder(xbi_ap, j + 1, "wie", "bie", "bi"),
                                       encoder(xbt_ap, j + 1, "wte", "bte", "bt"))

                    # D = (v_img + bv) - vt ; pooled = A*D + vt  (DVE)
                    dd = wp.tile([128, 2, T], BF, tag="dd", name="dd")
                    for m in range(2):
                        if zero_bias:
                            nc.vector.tensor_tensor(
                                dd[:, m, :], ps_vi[m][:], vt[:, m, :], op=OP.subtract
                            )
                        else:
                            nc.vector.scalar_tensor_tensor(
                                dd[:, m, :],
                                ps_vi[m][:],
                                w["bv"][:, m : m + 1],
                                vt[:, m, :],
                                op0=OP.add,
                                op1=OP.subtract,
                            )
                    pooled = wp.tile([128, 2, T], BF, tag="pooled", name="pooled")
                    tmp = wp.tile([128, 2, T], BF, tag="denc", name="tmp")
                    for m in range(2):
                        nc.vector.tensor_tensor(
                            tmp[:, m, :], ps_a[m][:], dd[:, m, :], op=OP.mult
                        )
                        nc.vector.tensor_tensor(
                            pooled[:, m, :], tmp[:, m, :], vt[:, m, :], op=OP.add
                        )

                    # fused = pooled @ Wof + bof
                    fused = wp.tile([128, 4, T], BF, tag="fused", name="fused")
                    proj_512(fused, pooled, "wof", "bof", "mix")
                    mlp_head(fused, dram["outb"][:], j)

            # ---------- single-modality pipelines ----------
            if ni:
                xii_ap = dram["xi_img"][:].rearrange("(k p) n -> p k n", p=128)
                only_pipe(xii_ap, dram["outi"][:], ni // T, "wie", "bie", "wip", "bip", "oi")
            if nt:
                xtt_ap = dram["xt_txt"][:].rearrange("(k p) n -> p k n", p=128)
                only_pipe(xtt_ap, dram["outt"][:], nt // T, "wte", "bte", "wtp", "btp", "ot")

    nc.compile()
    return nc


def _prep_weights(inp):
    """Host-side weight prep: fold/merge/transpose into the device layouts."""
    f32 = np.float32
    q = (inp["fusion_query"].reshape(1, H).astype(f32) @ inp["Wq"] + inp["bq"]).reshape(
        NH, HD
    )
    wkeff = np.zeros((H, NH), f32)
    for h in range(NH):
        wkeff[:, h] = inp["Wk"][:, h * HD : (h + 1) * HD] @ q[h] / np.sqrt(HD)
    wof = inp["Wo"].astype(f32) @ inp["W_fp"]
    bof = inp["bo"].astype(f32) @ inp["W_fp"] + inp["b_fp"]
    emat = np.zeros((NH, H), f32)
    for h in range(NH):
        emat[h, h * HD : (h + 1) * HD] = 1.0

    def ktile(a, kt):  # [K, M] -> [128, kt, M]
        return np.ascontiguousarray(
            a.reshape(kt, 128, a.shape[1]).transpose(1, 0, 2)
        )

    bf = ml_dtypes.bfloat16
    out = {
        "wie": ktile(inp["W_ie"], 4).astype(bf),
        "wte": ktile(inp["W_te"], 4).astype(bf),
        "wv": ktile(inp["Wv"], 2).astype(bf),
        "wkeff": ktile(wkeff, 2).astype(bf),
        "emat": np.ascontiguousarray(
            emat.reshape(NH, 2, 128)
        ).astype(bf),
        "wof": ktile(wof, 2).astype(bf),
        "wip": ktile(inp["W_ip"], 2).astype(bf),
        "wtp": ktile(inp["W_tp"], 2).astype(bf),
        "wc1": ktile(inp["Wc1"], 4).astype(bf),
        "wc2": ktile(inp["Wc2"], 2).astype(bf),
        "bie": np.ascontiguousarray(inp["b_ie"].reshape(2, 128).T).astype(f32),
        "bte": np.ascontiguousarray(inp["b_te"].reshape(2, 128).T).astype(f32),
        "bv": np.ascontiguousarray(inp["bv"].reshape(2, 128).T).astype(f32),
        "bof": np.ascontiguousarray(bof.reshape(4, 128).T).astype(f32),
        "bip": np.ascontiguousarray(inp["b_ip"].reshape(4, 128).T).astype(f32),
        "btp": np.ascontiguousarray(inp["b_tp"].reshape(4, 128).T).astype(f32),
        "bc1": np.ascontiguousarray(inp["bc1"].reshape(2, 128).T).astype(f32),
        "bc2": np.ascontiguousarray(
            np.pad(inp["bc2"].astype(f32), (0, 128 - NCLS)).reshape(128, 1)
        ),
    }
    return out


def _split_pad(idx, rng=None):
    """Split index array across cores evenly; pad each core's slice to a
    multiple of T with -1. Returns list of per-core padded index arrays
    (all the same length)."""
    per = [idx[c::NCORES] for c in range(NCORES)]
    n = max(len(p) for p in per)
    npad = ((n + T - 1) // T) * T if n else 0
    out = []
    for p in per:
        a = np.full(npad, -1, dtype=np.int64)
        a[: len(p)] = p
        out.append(a)
    return out


def _gather_ft(x_bf, idx):
    """Rows idx of x (with -1 -> zero row), as tile-contiguous
    [ntiles, D, T] bf16 (feature-major within each tile)."""
    n = len(idx)
    d = x_bf.shape[1]
    out = np.zeros((n, d), dtype=x_bf.dtype)
    valid = idx >= 0
    out[valid] = x_bf[idx[valid]]
    return np.ascontiguousarray(out.T)


def _ntff_hook():
    """Build the (output_dir, device_ids) -> contextmanager NTFF profile
    hook directly via ctypes on the axon PJRT .so (the image's antenv lacks
    axon_hooks, so the boot-time registration was skipped)."""
    import ctypes
    import contextlib

    so_path = "/opt/axon/libaxon_pjrt.so"
    lib = ctypes.CDLL(so_path)
    if not hasattr(lib, "axon_start_nrt_profile"):
        return None
    lib.axon_start_nrt_profile.argtypes = [
        ctypes.POINTER(ctypes.c_int64),
        ctypes.c_size_t,
    ]
    lib.axon_start_nrt_profile.restype = ctypes.c_int64
    lib.axon_stop_nrt_profile.argtypes = [ctypes.c_char_p]
    lib.axon_stop_nrt_profile.restype = ctypes.c_int64

    @contextlib.contextmanager
    def _hook(output_dir, device_ids):
        import jax

        jax.devices()
        if device_ids:
            ids = (ctypes.c_int64 * len(device_ids))(*device_ids)
            rc = lib.axon_start_nrt_profile(ids, len(device_ids))
        else:
            rc = lib.axon_start_nrt_profile(None, 0)
        if rc != 0:
            raise RuntimeError(f"axon_start_nrt_profile rc={rc}")
        try:
            yield
        finally:
            n = lib.axon_stop_nrt_profile(str(output_dir).encode())
            print(f"profile: {n} file(s) written to {output_dir}", file=sys.stderr)

    return _hook


def _profiled_run(nc, in_maps):
    """Run via PJRT with NTFF profiling; parse exec_time_ns from the trace."""
    import tempfile
    import glob as _glob

    from concourse import bass2jax
    from concourse._compat import FishPath
    import gauge.profiler

    hook = _ntff_hook()
    tmpdir = tempfile.mkdtemp(prefix="aecf_prof_")
    if hook is None:
        results = bass2jax.run_bass_via_pjrt(nc, in_maps, n_cores=NCORES)
        return results, None, None
    with hook(tmpdir, [0]):
        results = bass2jax.run_bass_via_pjrt(nc, in_maps, n_cores=NCORES)
    ntffs = _glob.glob(os.path.join(tmpdir, "*_body*.ntff"))
    if not ntffs:
        print(f"no NTFFs in {tmpdir}: {sorted(os.listdir(tmpdir))}", file=sys.stderr)
        return results, None, None
    prof = gauge.profiler.Profile(
        profile_path=FishPath(tmpdir),
        kernel_dev_mode=True,
        profile_on_exit=False,
        bass_kernel=nc.m,
        offline_processing=True,
        fname="*_body*",
        metadata={},
    )
    try:
        pres = prof.to_perfetto(model_index=(0,))
        exec_ns = pres[0].exec_time_ns if pres else None
        pjson = prof.json_path(0).path if pres else None
    except Exception as e:
        print(f"profile parse failed: {e}", file=sys.stderr)
        return results, None, None
    return results, exec_ns, pjson


def kernel(**inputs):
    global LAST_EXEC_NS, LAST_PROFILE
    img = np.asarray(inputs["image_features"], dtype=np.float32)
    txt = np.asarray(inputs["text_features"], dtype=np.float32)

    pres_i = np.linalg.norm(img, axis=1) > 1e-6
    pres_t = np.linalg.norm(txt, axis=1) > 1e-6
    both = pres_i & pres_t
    oi = pres_i & ~pres_t
    ot = ~pres_i & pres_t
    none = ~pres_i & ~pres_t

    idx_b = _split_pad(np.nonzero(both)[0])
    idx_i = _split_pad(np.nonzero(oi)[0])
    idx_t = _split_pad(np.nonzero(ot)[0])
    nb, ni, nt = len(idx_b[0]), len(idx_i[0]), len(idx_t[0])

    bias_names = ("b_ie", "b_te", "bv", "bo", "b_fp", "b_ip", "b_tp", "bc1", "bc2")
    zero_bias = all(not np.any(np.asarray(inputs[n])) for n in bias_names)
    key = (nb, ni, nt, zero_bias)
    if key not in _GRAPH_CACHE:
        _GRAPH_CACHE[key] = _build_graph(nb, ni, nt, zero_bias)
    nc = _GRAPH_CACHE[key]

    wmap = _prep_weights(inputs)
    bf = ml_dtypes.bfloat16
    img_bf = img.astype(bf)
    txt_bf = txt.astype(bf)

    in_maps = []
    for c in range(NCORES):
        m = dict(wmap)
        if nb:
            m["xb_img"] = _gather_ft(img_bf, idx_b[c])
            m["xb_txt"] = _gather_ft(txt_bf, idx_b[c])
        if ni:
            m["xi_img"] = _gather_ft(img_bf, idx_i[c])
        if nt:
            m["xt_txt"] = _gather_ft(txt_bf, idx_t[c])
        in_maps.append(m)

    trace = bool(int(os.environ.get("KERNEL_PROFILE", "0")))
    if trace:
        results, exec_ns, prof_json = _profiled_run(nc, in_maps)
        LAST_EXEC_NS = exec_ns
        LAST_PROFILE = prof_json

        class _R:
            pass

        res = _R()
        res.results = results
    else:
        res = run_bass_kernel_spmd(nc, in_maps, core_ids=list(range(NCORES)))
        LAST_EXEC_NS = None
        LAST_PROFILE = None

    logits = np.empty((img.shape[0], NCLS), dtype=np.float32)
    for c in range(NCORES):
        r = res.results[c]
        for name, idx in (("outb", idx_b[c]), ("outi", idx_i[c]), ("outt", idx_t[c])):
            if name in r:
                valid = idx >= 0
                logits[idx[valid]] = r[name].T[valid]

    if none.any():
        # reference: fused = 0 -> logits = relu(bc1) @ Wc2 + bc2 (constant)
        row = (
            np.maximum(inputs["bc1"].astype(np.float32), 0.0) @ inputs["Wc2"]
            + inputs["bc2"]
        )
        logits[none] = row
    return logits



# revision 3
# speedup vs baseline: 1.4431x; 1.4431x over previous
"""AECF multimodal fusion kernel for 8 TRN2 NeuronCores.

Strategy:
  - Host-side routing (part of sharding): rows are sorted into three branch
    groups (both modalities present / only-image / only-text) using the same
    norm>1e-6 predicate as the reference. Each group is dealt evenly across
    the 8 cores and padded to a tile multiple; the NEFF is compiled with the
    actual per-core group sizes (compile happens inside kernel(), after the
    inputs are known), so the graph is static and identical on all cores.
  - Tile-major feature-major layout: inputs are shipped as [ntile, 128, 4, T]
    bf16 blocks (4KB contiguous per partition per tile), outputs leave as
    [ntile, 80, T] f32 blocks.
  - bf16 storage/compute, f32 PSUM accumulation.
  - Algebra: scores only enter through a 2-way softmax, so
    attn_img = sigmoid(s_img - s_txt) with s = enc @ Wk_eff,
    Wk_eff[:, h] = Wk[:, 64h:64h+64] @ q[h] / 8 (bk cancels in the diff).
    There is no nonlinearity between the post-attention projection and the
    classifier's first layer, so Wo, W_fp, Wc1 merge into Wofc1 [256,256]:
      h1pre = pooled @ Wofc1 + bh1
            = enc_t @ (Wv@Wofc1) + (A * v_d) @ Wofc1 + bh1
    with v_d = (enc_i - enc_t) @ Wv and A = head-broadcast(attn) (PE matmul
    with a 0/1 expander).  v_txt is never materialized.  Only-branches use
    Wipc1 = W_ip@Wc1 / Wtpc1 = W_tp@Wc1.
  - Depth-3 software pipeline per both-tile, ordered so the PE queue (the
    bottleneck engine) never waits on ACT/DVE round trips; PSUM tiles span
    2 banks so evacuations/relus are single ops over [128, 1024].
"""

import os
import sys

if "/opt/trn_rl_repo" not in sys.path:
    sys.path.insert(0, "/opt/trn_rl_repo")

import numpy as np
import ml_dtypes

import concourse.bass as bass
import concourse.bacc as bacc
import concourse.tile as tile
from concourse import mybir
from concourse.bass_utils import run_bass_kernel_spmd

BF = mybir.dt.bfloat16
F32 = mybir.dt.float32
AF = mybir.ActivationFunctionType
OP = mybir.AluOpType

H = 256
ID = 512
TD = 512
NCLS = 80
NH = 4
HD = 64
B = 131072
NCORES = 8
T = 512  # batch-tile (free-dim) size; one psum bank of f32 per 128-chunk

LAST_EXEC_NS = None
LAST_PROFILE = None

_GRAPH_CACHE = {}


def _build_graph(nb, ni, nt, zero_bias):
    """Build the SPMD graph for per-core group column counts nb/ni/nt
    (each a multiple of T, possibly 0)."""
    nc = bacc.Bacc()
    nbt, nit, ntt = nb // T, ni // T, nt // T

    # ---- DRAM I/O ----
    dram = {}
    if nbt:
        dram["xb_img"] = nc.dram_tensor("xb_img", [nbt, 128, 4, T], BF, kind="ExternalInput")
        dram["xb_txt"] = nc.dram_tensor("xb_txt", [nbt, 128, 4, T], BF, kind="ExternalInput")
        dram["outb"] = nc.dram_tensor("outb", [nbt, NCLS, T], F32, kind="ExternalOutput")
    if nit:
        dram["xi_img"] = nc.dram_tensor("xi_img", [nit, 128, 4, T], BF, kind="ExternalInput")
        dram["outi"] = nc.dram_tensor("outi", [nit, NCLS, T], F32, kind="ExternalOutput")
    if ntt:
        dram["xt_txt"] = nc.dram_tensor("xt_txt", [ntt, 128, 4, T], BF, kind="ExternalInput")
        dram["outt"] = nc.dram_tensor("outt", [ntt, NCLS, T], F32, kind="ExternalOutput")

    wspec = {
        "wie": ([128, 4, H], BF),
        "wte": ([128, 4, H], BF),
        "wkeff": ([128, 2, NH], BF),
        "emat": ([NH, 2, 128], BF),
        "wv": ([128, 2, H], BF),
        "wvofc1": ([128, 2, H], BF),
        "wofc1": ([128, 2, H], BF),
        "wipc1": ([128, 2, H], BF),
        "wtpc1": ([128, 2, H], BF),
        "wc2": ([128, 2, NCLS], BF),
        "bie": ([128, 2], F32),
        "bte": ([128, 2], F32),
        "bh1b": ([128, 2], F32),
        "bh1i": ([128, 2], F32),
        "bh1t": ([128, 2], F32),
        "bc2": ([128, 1], F32),
    }
    for name, (shape, dt) in wspec.items():
        dram[name] = nc.dram_tensor(name, shape, dt, kind="ExternalInput")

    with tile.TileContext(nc) as tc:
        with (
            tc.tile_pool(name="wpool", bufs=1) as wpool,
            tc.tile_pool(name="work", bufs=2) as wp,
            tc.tile_pool(name="psum", bufs=1, space="PSUM") as pp,
        ):
            w = {}
            for name, (shape, dt) in wspec.items():
                w[name] = wpool.tile(shape, dt, tag=name, name=name)
                nc.gpsimd.dma_start(w[name][:], dram[name][:])

            def relu_evac(dst, ps, bias, tag2):
                """psum [128,2,T] -> sbuf bf16 with relu (+bias per m-half)."""
                if zero_bias:
                    nc.scalar.activation(dst[:, :, :], ps[:, :, :], AF.Relu)
                else:
                    for m in range(2):
                        nc.scalar.activation(
                            dst[:, m, :], ps[:, m, :], AF.Relu,
                            bias=w[tag2][:, m : m + 1],
                        )

            # ================= both-modality pipeline =================
            # Stages per tile i:
            #   S1: input DMAs      S2: encoders (PE 16) + relus + denc
            #   S3a: wkeff (PE 2) + sigmoid     S3b: vd (PE 4) + emat (PE 2)
            #        + vd evac + tmp = A*vd
            #   S4: h1pre (PE 8: enc_t@Wvofc1 + tmp@Wofc1) + h1 relu
            #   S5: wc2 (PE 2) + out evac + out DMA
            # Iteration i emits: S3a(i), S2(i+1), S3b(i), S4(i-1), S5(i-2),
            # S1(i+2) so the in-order PE queue never waits on ACT/DVE.
            X, ENC, DNC, ATT, TMP, H1 = {}, {}, {}, {}, {}, {}

            def b_s1(i):
                xi = wp.tile([128, 4, T], BF, tag="xi", bufs=3, name="xi")
                xt = wp.tile([128, 4, T], BF, tag="xt", bufs=3, name="xt")
                nc.sync.dma_start(xi[:], dram["xb_img"][i])
                nc.gpsimd.dma_start(xt[:], dram["xb_txt"][i])
                X[i] = (xi, xt)

            def b_s2(i):
                xi, xt = X.pop(i)
                pei = pp.tile([128, 2, T], F32, tag="ps_big", bufs=3, name="pei")
                for m in range(2):
                    for k in range(4):
                        nc.tensor.matmul(
                            pei[:, m, :], w["wie"][:, k, m * 128 : (m + 1) * 128],
                            xi[:, k, :], start=(k == 0), stop=(k == 3),
                        )
                enci = wp.tile([128, 2, T], BF, tag="enci", bufs=2, name="enci")
                relu_evac(enci, pei, None, "bie")
                pet = pp.tile([128, 2, T], F32, tag="ps_big", bufs=3, name="pet")
                for m in range(2):
                    for k in range(4):
                        nc.tensor.matmul(
                            pet[:, m, :], w["wte"][:, k, m * 128 : (m + 1) * 128],
                            xt[:, k, :], start=(k == 0), stop=(k == 3),
                        )
                enct = wp.tile([128, 2, T], BF, tag="enct", bufs=3, name="enct")
                relu_evac(enct, pet, None, "bte")
                denc = wp.tile([128, 2, T], BF, tag="denc", bufs=3, name="denc")
                nc.vector.tensor_tensor(denc[:, :, :], enci[:, :, :], enct[:, :, :],
                                        op=OP.subtract)
                ENC[i] = enct
                DNC[i] = denc

            def b_s3a(i):
                d = DNC[i]
                pss = pp.tile([128, T], F32, tag="ps_small", bufs=2, name="pss")
                for k in range(2):
                    nc.tensor.matmul(pss[:NH, :], w["wkeff"][:, k, :], d[:, k, :],
                                     start=(k == 0), stop=(k == 1))
                att = wp.tile([NH, T], BF, tag="att", bufs=3, name="att")
                nc.scalar.activation(att[:], pss[:NH, :], AF.Sigmoid)
                ATT[i] = att

            def b_s3b(i):
                d = DNC.pop(i)
                att = ATT.pop(i)
                psv = pp.tile([128, 2, T], F32, tag="ps_big", bufs=3, name="psv")
                for m in range(2):
                    for k in range(2):
                        nc.tensor.matmul(
                            psv[:, m, :], w["wv"][:, k, m * 128 : (m + 1) * 128],
                            d[:, k, :], start=(k == 0), stop=(k == 1),
                        )
                psa = pp.tile([128, 2, T], F32, tag="ps_big", bufs=3, name="psa")
                for m in range(2):
                    nc.tensor.matmul(psa[:, m, :], w["emat"][:, m, :], att[:],
                                     start=True, stop=True)
                vd = wp.tile([128, 2, T], BF, tag="vd", bufs=2, name="vd")
                nc.vector.tensor_copy(vd[:, :, :], psv[:, :, :])
                tmp = wp.tile([128, 2, T], BF, tag="tmp", bufs=3, name="tmp")
                nc.vector.tensor_tensor(tmp[:, :, :], psa[:, :, :], vd[:, :, :],
                                        op=OP.mult)
                TMP[i] = tmp

            def b_s4(i):
                enct = ENC.pop(i)
                tmp = TMP.pop(i)
                psh = pp.tile([128, 2, T], F32, tag="ps_big", bufs=3, name="psh")
                for m in range(2):
                    ms = slice(m * 128, (m + 1) * 128)
                    nc.tensor.matmul(psh[:, m, :], w["wvofc1"][:, 0, ms],
                                     enct[:, 0, :], start=True, stop=False)
                    nc.tensor.matmul(psh[:, m, :], w["wvofc1"][:, 1, ms],
                                     enct[:, 1, :], start=False, stop=False)
                    nc.tensor.matmul(psh[:, m, :], w["wofc1"][:, 0, ms],
                                     tmp[:, 0, :], start=False, stop=False)
                    nc.tensor.matmul(psh[:, m, :], w["wofc1"][:, 1, ms],
                                     tmp[:, 1, :], start=False, stop=True)
                h1 = wp.tile([128, 2, T], BF, tag="h1", bufs=3, name="h1")
                relu_evac(h1, psh, None, "bh1b")
                H1[i] = h1

            def b_s5(i):
                h1 = H1.pop(i)
                pso = pp.tile([128, T], F32, tag="ps_small", bufs=2, name="pso")
                for k in range(2):
                    nc.tensor.matmul(pso[:NCLS, :], w["wc2"][:, k, :], h1[:, k, :],
                                     start=(k == 0), stop=(k == 1))
                osb = wp.tile([NCLS, T], F32, tag="osb", bufs=3, name="osb")
                if zero_bias:
                    nc.vector.tensor_copy(osb[:], pso[:NCLS, :])
                else:
                    nc.vector.tensor_scalar_add(osb[:], pso[:NCLS, :],
                                                w["bc2"][:NCLS, :])
                nc.sync.dma_start(dram["outb"][i], osb[:])

            if nbt:
                b_s1(0)
                if nbt > 1:
                    b_s1(1)
                b_s2(0)
                for i in range(nbt + 2):
                    if i < nbt:
                        b_s3a(i)
                    if i + 1 < nbt:
                        b_s2(i + 1)
                    if i < nbt:
                        b_s3b(i)
                    if 0 <= i - 1 < nbt:
                        b_s4(i - 1)
                    if 0 <= i - 2 < nbt:
                        b_s5(i - 2)
                    if i + 2 < nbt:
                        b_s1(i + 2)

            # ================= single-modality pipelines =================
            # out = relu(enc @ Wpc1 + bh1) @ Wc2 + bc2, enc = relu(x@We+be)
            def only_pipe(x_dram, out_dram, ntiles, wenc, benc, wproj, bproj):
                XO, ENO, HO = {}, {}, {}

                def o_s1(j):
                    xo = wp.tile([128, 4, T], BF, tag="xi", bufs=3, name="xo")
                    nc.sync.dma_start(xo[:], x_dram[j])
                    XO[j] = xo

                def o_s2(j):
                    xo = XO.pop(j)
                    pe = pp.tile([128, 2, T], F32, tag="ps_big", bufs=3, name="peo")
                    for m in range(2):
                        for k in range(4):
                            nc.tensor.matmul(
                                pe[:, m, :], w[wenc][:, k, m * 128 : (m + 1) * 128],
                                xo[:, k, :], start=(k == 0), stop=(k == 3),
                            )
                    eno = wp.tile([128, 2, T], BF, tag="enct", bufs=3, name="eno")
                    relu_evac(eno, pe, None, benc)
                    ENO[j] = eno

                def o_s3(j):
                    eno = ENO.pop(j)
                    psh = pp.tile([128, 2, T], F32, tag="ps_big", bufs=3, name="psho")
                    for m in range(2):
                        ms = slice(m * 128, (m + 1) * 128)
                        for k in range(2):
                            nc.tensor.matmul(psh[:, m, :], w[wproj][:, k, ms],
                                             eno[:, k, :], start=(k == 0),
                                             stop=(k == 1))
                    h1 = wp.tile([128, 2, T], BF, tag="h1", bufs=3, name="h1o")
                    if zero_bias:
                        nc.vector.tensor_scalar_max(h1[:, :, :], psh[:, :, :], 0.0)
                    else:
                        for m in range(2):
                            nc.scalar.activation(
                                h1[:, m, :], psh[:, m, :], AF.Relu,
                                bias=w[bproj][:, m : m + 1],
                            )
                    HO[j] = h1

                def o_s4(j):
                    h1 = HO.pop(j)
                    pso = pp.tile([128, T], F32, tag="ps_small", bufs=2, name="psoo")
                    for k in range(2):
                        nc.tensor.matmul(pso[:NCLS, :], w["wc2"][:, k, :],
                                         h1[:, k, :], start=(k == 0), stop=(k == 1))
                    osb = wp.tile([NCLS, T], F32, tag="osb", bufs=3, name="osbo")
                    if zero_bias:
                        nc.vector.tensor_copy(osb[:], pso[:NCLS, :])
                    else:
                        nc.vector.tensor_scalar_add(osb[:], pso[:NCLS, :],
                                                    w["bc2"][:NCLS, :])
                    nc.gpsimd.dma_start(out_dram[j], osb[:])

                o_s1(0)
                if ntiles > 1:
                    o_s1(1)
                o_s2(0)
                for j in range(ntiles + 1):
                    if j + 1 < ntiles:
                        o_s2(j + 1)
                    if j < ntiles:
                        o_s3(j)
                    if 0 <= j - 1 < ntiles:
                        o_s4(j - 1)
                    if j + 2 < ntiles:
                        o_s1(j + 2)

            if nit:
                only_pipe(dram["xi_img"], dram["outi"], nit, "wie", "bie", "wipc1", "bh1i")
            if ntt:
                only_pipe(dram["xt_txt"], dram["outt"], ntt, "wte", "bte", "wtpc1", "bh1t")

    nc.compile()
    return nc


def _prep_weights(inp):
    """Host-side weight prep: fold/merge/transpose into the device layouts."""
    f32 = np.float32
    q = (inp["fusion_query"].reshape(1, H).astype(f32) @ inp["Wq"] + inp["bq"]).reshape(
        NH, HD
    )
    wkeff = np.zeros((H, NH), f32)
    for h in range(NH):
        wkeff[:, h] = inp["Wk"][:, h * HD : (h + 1) * HD] @ q[h] / np.sqrt(HD)
    wof = inp["Wo"].astype(f32) @ inp["W_fp"]
    bof = inp["bo"].astype(f32) @ inp["W_fp"] + inp["b_fp"]
    wofc1 = wof @ inp["Wc1"]
    wvofc1 = inp["Wv"].astype(f32) @ wofc1
    bh1b = inp["bv"].astype(f32) @ wofc1 + bof @ inp["Wc1"] + inp["bc1"]
    wipc1 = inp["W_ip"].astype(f32) @ inp["Wc1"]
    bh1i = inp["b_ip"].astype(f32) @ inp["Wc1"] + inp["bc1"]
    wtpc1 = inp["W_tp"].astype(f32) @ inp["Wc1"]
    bh1t = inp["b_tp"].astype(f32) @ inp["Wc1"] + inp["bc1"]
    emat = np.zeros((NH, H), f32)
    for h in range(NH):
        emat[h, h * HD : (h + 1) * HD] = 1.0

    def ktile(a, kt):  # [K, M] -> [128, kt, M]
        return np.ascontiguousarray(
            a.reshape(kt, 128, a.shape[1]).transpose(1, 0, 2)
        )

    bf = ml_dtypes.bfloat16
    out = {
        "wie": ktile(inp["W_ie"], 4).astype(bf),
        "wte": ktile(inp["W_te"], 4).astype(bf),
        "wkeff": ktile(wkeff, 2).astype(bf),
        "emat": np.ascontiguousarray(emat.reshape(NH, 2, 128)).astype(bf),
        "wv": ktile(inp["Wv"].astype(f32), 2).astype(bf),
        "wvofc1": ktile(wvofc1, 2).astype(bf),
        "wofc1": ktile(wofc1, 2).astype(bf),
        "wipc1": ktile(wipc1, 2).astype(bf),
        "wtpc1": ktile(wtpc1, 2).astype(bf),
        "wc2": ktile(inp["Wc2"].astype(f32), 2).astype(bf),
        "bie": np.ascontiguousarray(inp["b_ie"].reshape(2, 128).T).astype(f32),
        "bte": np.ascontiguousarray(inp["b_te"].reshape(2, 128).T).astype(f32),
        "bh1b": np.ascontiguousarray(bh1b.reshape(2, 128).T).astype(f32),
        "bh1i": np.ascontiguousarray(bh1i.reshape(2, 128).T).astype(f32),
        "bh1t": np.ascontiguousarray(bh1t.reshape(2, 128).T).astype(f32),
        "bc2": np.ascontiguousarray(
            np.pad(inp["bc2"].astype(f32), (0, 128 - NCLS)).reshape(128, 1)
        ),
    }
    return out


def _split_pad(idx):
    """Split index array across cores evenly; pad each core's slice to a
    multiple of T with -1. Returns list of per-core padded index arrays
    (all the same length)."""
    per = [idx[c::NCORES] for c in range(NCORES)]
    n = max(len(p) for p in per)
    npad = ((n + T - 1) // T) * T if n else 0
    out = []
    for p in per:
        a = np.full(npad, -1, dtype=np.int64)
        a[: len(p)] = p
        out.append(a)
    return out


def _gather_tiles(x_bf, idx):
    """Rows idx of x (with -1 -> zero row), as tile-major blocks
    [ntiles, 128, 4, T] bf16: block[j, p, k, c] = x[idx[j*T+c], k*128+p]."""
    n = len(idx)
    d = x_bf.shape[1]
    g = np.zeros((n, d), dtype=x_bf.dtype)
    valid = idx >= 0
    g[valid] = x_bf[idx[valid]]
    return np.ascontiguousarray(g.reshape(n // T, T, 4, 128).transpose(0, 3, 2, 1))


def _ntff_hook():
    """Build the (output_dir, device_ids) -> contextmanager NTFF profile
    hook directly via ctypes on the axon PJRT .so (the image's antenv lacks
    axon_hooks, so the boot-time registration was skipped)."""
    import ctypes
    import contextlib

    so_path = "/opt/axon/libaxon_pjrt.so"
    lib = ctypes.CDLL(so_path)
    if not hasattr(lib, "axon_start_nrt_profile"):
        return None
    lib.axon_start_nrt_profile.argtypes = [
        ctypes.POINTER(ctypes.c_int64),
        ctypes.c_size_t,
    ]
    lib.axon_start_nrt_profile.restype = ctypes.c_int64
    lib.axon_stop_nrt_profile.argtypes = [ctypes.c_char_p]
    lib.axon_stop_nrt_profile.restype = ctypes.c_int64

    @contextlib.contextmanager
    def _hook(output_dir, device_ids):
        import jax

        jax.devices()
        if device_ids:
            ids = (ctypes.c_int64 * len(device_ids))(*device_ids)
            rc = lib.axon_start_nrt_profile(ids, len(device_ids))
        else:
            rc = lib.axon_start_nrt_profile(None, 0)
        if rc != 0:
            raise RuntimeError(f"axon_start_nrt_profile rc={rc}")
        try:
            yield
        finally:
            n = lib.axon_stop_nrt_profile(str(output_dir).encode())
            print(f"profile: {n} file(s) written to {output_dir}", file=sys.stderr)

    return _hook


def _profiled_run(nc, in_maps):
    """Run via PJRT with NTFF profiling; parse exec_time_ns from the trace."""
    import tempfile
    import glob as _glob

    from concourse import bass2jax
    from concourse._compat import FishPath
    import gauge.profiler

    hook = _ntff_hook()
    tmpdir = tempfile.mkdtemp(prefix="aecf_prof_")
    if hook is None:
        results = bass2jax.run_bass_via_pjrt(nc, in_maps, n_cores=NCORES)
        return results, None, None
    with hook(tmpdir, [0]):
        results = bass2jax.run_bass_via_pjrt(nc, in_maps, n_cores=NCORES)
    ntffs = _glob.glob(os.path.join(tmpdir, "*_body*.ntff"))
    if not ntffs:
        print(f"no NTFFs in {tmpdir}: {sorted(os.listdir(tmpdir))}", file=sys.stderr)
        return results, None, None
    prof = gauge.profiler.Profile(
        profile_path=FishPath(tmpdir),
        kernel_dev_mode=True,
        profile_on_exit=False,
        bass_kernel=nc.m,
        offline_processing=True,
        fname="*_body*",
        metadata={},
    )
    try:
        pres = prof.to_perfetto(model_index=(0,))
        exec_ns = pres[0].exec_time_ns if pres else None
        pjson = prof.json_path(0).path if pres else None
    except Exception as e:
        print(f"profile parse failed: {e}", file=sys.stderr)
        return results, None, None
    return results, exec_ns, pjson


def kernel(**inputs):
    global LAST_EXEC_NS, LAST_PROFILE
    img = np.asarray(inputs["image_features"], dtype=np.float32)
    txt = np.asarray(inputs["text_features"], dtype=np.float32)

    pres_i = np.linalg.norm(img, axis=1) > 1e-6
    pres_t = np.linalg.norm(txt, axis=1) > 1e-6
    both = pres_i & pres_t
    oi = pres_i & ~pres_t
    ot = ~pres_i & pres_t
    none = ~pres_i & ~pres_t

    idx_b = _split_pad(np.nonzero(both)[0])
    idx_i = _split_pad(np.nonzero(oi)[0])
    idx_t = _split_pad(np.nonzero(ot)[0])
    nb, ni, nt = len(idx_b[0]), len(idx_i[0]), len(idx_t[0])

    bias_names = ("b_ie", "b_te", "bv", "bo", "b_fp", "b_ip", "b_tp", "bc1", "bc2")
    zero_bias = all(not np.any(np.asarray(inputs[n])) for n in bias_names)
    key = (nb, ni, nt, zero_bias)
    if key not in _GRAPH_CACHE:
        _GRAPH_CACHE[key] = _build_graph(nb, ni, nt, zero_bias)
    nc = _GRAPH_CACHE[key]

    wmap = _prep_weights(inputs)
    bf = ml_dtypes.bfloat16
    img_bf = img.astype(bf)
    txt_bf = txt.astype(bf)

    in_maps = []
    for c in range(NCORES):
        m = dict(wmap)
        if nb:
            m["xb_img"] = _gather_tiles(img_bf, idx_b[c])
            m["xb_txt"] = _gather_tiles(txt_bf, idx_b[c])
        if ni:
            m["xi_img"] = _gather_tiles(img_bf, idx_i[c])
        if nt:
            m["xt_txt"] = _gather_tiles(txt_bf, idx_t[c])
        in_maps.append(m)

    trace = bool(int(os.environ.get("KERNEL_PROFILE", "0")))
    if trace:
        results, exec_ns, prof_json = _profiled_run(nc, in_maps)
        LAST_EXEC_NS = exec_ns
        LAST_PROFILE = prof_json

        class _R:
            pass

        res = _R()
        res.results = results
    else:
        res = run_bass_kernel_spmd(nc, in_maps, core_ids=list(range(NCORES)))
        LAST_EXEC_NS = None
        LAST_PROFILE = None

    logits = np.empty((img.shape[0], NCLS), dtype=np.float32)
    for c in range(NCORES):
        r = res.results[c]
        for name, idx in (("outb", idx_b[c]), ("outi", idx_i[c]), ("outt", idx_t[c])):
            if name in r:
                valid = idx >= 0
                flat = np.ascontiguousarray(r[name].transpose(0, 2, 1)).reshape(
                    -1, NCLS
                )
                logits[idx[valid]] = flat[valid]

    if none.any():
        # reference: fused = 0 -> logits = relu(bc1) @ Wc2 + bc2 (constant)
        row = (
            np.maximum(inputs["bc1"].astype(np.float32), 0.0) @ inputs["Wc2"]
            + inputs["bc2"]
        )
        logits[none] = row
    return logits


# revision 9
# speedup vs baseline: 1.4510x; 1.0055x over previous
"""AECF multimodal fusion kernel for 8 TRN2 NeuronCores.

Strategy:
  - Host-side routing (part of sharding): rows are sorted into three branch
    groups (both modalities present / only-image / only-text) using the same
    norm>1e-6 predicate as the reference. Each group is dealt evenly across
    the 8 cores and padded to a tile multiple; the NEFF is compiled with the
    actual per-core group sizes (compile happens inside kernel(), after the
    inputs are known), so the graph is static and identical on all cores.
  - Tile-major feature-major layout: inputs are shipped as [ntile, 128, 4, T]
    bf16 blocks (4KB contiguous per partition per tile), outputs leave as
    [ntile, 80, T] f32 blocks.
  - bf16 storage/compute, f32 PSUM accumulation.
  - Algebra: scores only enter through a 2-way softmax, so
    attn_img = sigmoid(s_img - s_txt) with s = enc @ Wk_eff,
    Wk_eff[:, h] = Wk[:, 64h:64h+64] @ q[h] / 8 (bk cancels in the diff).
    There is no nonlinearity between the post-attention projection and the
    classifier's first layer, so Wo, W_fp, Wc1 merge into Wofc1 [256,256]:
      h1pre = pooled @ Wofc1 + bh1
            = enc_t @ (Wv@Wofc1) + (A * v_d) @ Wofc1 + bh1
    with v_d = (enc_i - enc_t) @ Wv and A = head-broadcast(attn) (PE matmul
    with a 0/1 expander).  v_txt is never materialized.  Only-branches use
    Wipc1 = W_ip@Wc1 / Wtpc1 = W_tp@Wc1.
  - Depth-3 software pipeline per both-tile, ordered so the PE queue (the
    bottleneck engine) never waits on ACT/DVE round trips; PSUM tiles span
    2 banks so evacuations/relus are single ops over [128, 1024].
"""

import os
import sys

if "/opt/trn_rl_repo" not in sys.path:
    sys.path.insert(0, "/opt/trn_rl_repo")

import numpy as np
import ml_dtypes

import concourse.bass as bass
import concourse.bacc as bacc
import concourse.tile as tile
from concourse import mybir
from concourse.bass_utils import run_bass_kernel_spmd

BF = mybir.dt.bfloat16
F32 = mybir.dt.float32
AF = mybir.ActivationFunctionType
OP = mybir.AluOpType

H = 256
ID = 512
TD = 512
NCLS = 80
NH = 4
HD = 64
B = 131072
NCORES = 8
T = 512  # batch-tile (free-dim) size; one psum bank of f32 per 128-chunk

LAST_EXEC_NS = None
LAST_PROFILE = None

_GRAPH_CACHE = {}


def _build_graph(nb, ni, nt, zero_bias):
    """Build the SPMD graph for per-core group column counts nb/ni/nt
    (each a multiple of T, possibly 0)."""
    nc = bacc.Bacc()
    nbt, nit, ntt = nb // T, ni // T, nt // T

    # ---- DRAM I/O ----
    dram = {}
    if nbt:
        dram["xb_img"] = nc.dram_tensor("xb_img", [nbt, 128, 4, T], BF, kind="ExternalInput")
        dram["xb_txt"] = nc.dram_tensor("xb_txt", [nbt, 128, 4, T], BF, kind="ExternalInput")
        dram["outb"] = nc.dram_tensor("outb", [nbt, NCLS, T], F32, kind="ExternalOutput")
    if nit:
        dram["xi_img"] = nc.dram_tensor("xi_img", [nit, 128, 4, T], BF, kind="ExternalInput")
        dram["outi"] = nc.dram_tensor("outi", [nit, NCLS, T], F32, kind="ExternalOutput")
    if ntt:
        dram["xt_txt"] = nc.dram_tensor("xt_txt", [ntt, 128, 4, T], BF, kind="ExternalInput")
        dram["outt"] = nc.dram_tensor("outt", [ntt, NCLS, T], F32, kind="ExternalOutput")

    wspec = {
        "wie": ([128, 4, H], BF),
        "wte": ([128, 4, H], BF),
        "wkeff": ([128, 2, NH], BF),
        "emat": ([NH, 2, 128], BF),
        "wv": ([128, 2, H], BF),
        "wvofc1": ([128, 2, H], BF),
        "wofc1": ([128, 2, H], BF),
        "wipc1": ([128, 2, H], BF),
        "wtpc1": ([128, 2, H], BF),
        "wc2": ([128, 2, NCLS], BF),
        "bie": ([128, 2], F32),
        "bte": ([128, 2], F32),
        "bh1b": ([128, 2], F32),
        "bh1i": ([128, 2], F32),
        "bh1t": ([128, 2], F32),
        "bc2": ([128, 1], F32),
    }
    for name, (shape, dt) in wspec.items():
        dram[name] = nc.dram_tensor(name, shape, dt, kind="ExternalInput")

    with tile.TileContext(nc) as tc:
        with (
            tc.tile_pool(name="wpool", bufs=1) as wpool,
            tc.tile_pool(name="work", bufs=2) as wp,
            tc.tile_pool(name="psum", bufs=1, space="PSUM") as pp,
        ):
            w = {}
            for wi, (name, (shape, dt)) in enumerate(wspec.items()):
                w[name] = wpool.tile(shape, dt, tag=name, name=name)
                eng = nc.gpsimd if wi % 2 == 0 else nc.scalar
                eng.dma_start(w[name][:], dram[name][:])

            def relu_evac(dst, ps, bias, tag2):
                """psum [128,2,T] -> sbuf bf16 with relu (+bias per m-half)."""
                if zero_bias:
                    nc.scalar.activation(dst[:, :, :], ps[:, :, :], AF.Relu)
                else:
                    for m in range(2):
                        nc.scalar.activation(
                            dst[:, m, :], ps[:, m, :], AF.Relu,
                            bias=w[tag2][:, m : m + 1],
                        )

            # ================= both-modality pipeline =================
            # Stages per tile i:
            #   S1: input DMAs      S2: encoders (PE 16) + relus + denc
            #   S3a: wkeff (PE 2) + sigmoid     S3b: vd (PE 4) + emat (PE 2)
            #        + vd evac + tmp = A*vd
            #   S4: h1pre (PE 8: enc_t@Wvofc1 + tmp@Wofc1) + h1 relu
            #   S5: wc2 (PE 2) + out evac + out DMA
            # Iteration i emits: S3a(i), S2(i+1), S3b(i), S4(i-1), S5(i-2),
            # S1(i+2) so the in-order PE queue never waits on ACT/DVE.
            X, ENC, DNC, ATT, TMP, H1 = {}, {}, {}, {}, {}, {}

            def b_s1(i):
                xi = wp.tile([128, 4, T], BF, tag="xi", bufs=4, name="xi")
                xt = wp.tile([128, 4, T], BF, tag="xt", bufs=4, name="xt")
                nc.sync.dma_start(xi[:], dram["xb_img"][i])
                nc.scalar.dma_start(xt[:], dram["xb_txt"][i])
                X[i] = (xi, xt)

            def b_s2(i):
                xi, xt = X.pop(i)
                pei = pp.tile([128, 2, T], F32, tag="ps_big", bufs=3, name="pei")
                for m in range(2):
                    for k in range(4):
                        nc.tensor.matmul(
                            pei[:, m, :], w["wie"][:, k, m * 128 : (m + 1) * 128],
                            xi[:, k, :], start=(k == 0), stop=(k == 3),
                        )
                enci = wp.tile([128, 2, T], BF, tag="enci", bufs=2, name="enci")
                relu_evac(enci, pei, None, "bie")
                pet = pp.tile([128, 2, T], F32, tag="ps_big", bufs=3, name="pet")
                for m in range(2):
                    for k in range(4):
                        nc.tensor.matmul(
                            pet[:, m, :], w["wte"][:, k, m * 128 : (m + 1) * 128],
                            xt[:, k, :], start=(k == 0), stop=(k == 3),
                        )
                enct = wp.tile([128, 2, T], BF, tag="enct", bufs=3, name="enct")
                relu_evac(enct, pet, None, "bte")
                denc = wp.tile([128, 2, T], BF, tag="denc", bufs=3, name="denc")
                nc.vector.tensor_tensor(denc[:, :, :], enci[:, :, :], enct[:, :, :],
                                        op=OP.subtract)
                ENC[i] = enct
                DNC[i] = denc

            def b_s3a(i):
                d = DNC[i]
                pss = pp.tile([128, T], F32, tag="ps_small", bufs=2, name="pss")
                for k in range(2):
                    nc.tensor.matmul(pss[:NH, :], w["wkeff"][:, k, :], d[:, k, :],
                                     start=(k == 0), stop=(k == 1))
                att = wp.tile([NH, T], BF, tag="att", bufs=3, name="att")
                nc.scalar.activation(att[:], pss[:NH, :], AF.Sigmoid)
                ATT[i] = att

            def b_s3b(i):
                d = DNC.pop(i)
                att = ATT.pop(i)
                psv = pp.tile([128, 2, T], F32, tag="ps_big", bufs=3, name="psv")
                for m in range(2):
                    for k in range(2):
                        nc.tensor.matmul(
                            psv[:, m, :], w["wv"][:, k, m * 128 : (m + 1) * 128],
                            d[:, k, :], start=(k == 0), stop=(k == 1),
                        )
                psa = pp.tile([128, 2, T], F32, tag="ps_big", bufs=3, name="psa")
                for m in range(2):
                    nc.tensor.matmul(psa[:, m, :], w["emat"][:, m, :], att[:],
                                     start=True, stop=True)
                vd = wp.tile([128, 2, T], BF, tag="vd", bufs=2, name="vd")
                nc.vector.tensor_copy(vd[:, :, :], psv[:, :, :])
                tmp = wp.tile([128, 2, T], BF, tag="tmp", bufs=3, name="tmp")
                nc.vector.tensor_tensor(tmp[:, :, :], psa[:, :, :], vd[:, :, :],
                                        op=OP.mult)
                TMP[i] = tmp

            def b_s4(i):
                enct = ENC.pop(i)
                tmp = TMP.pop(i)
                psh = pp.tile([128, 2, T], F32, tag="ps_big", bufs=3, name="psh")
                for m in range(2):
                    ms = slice(m * 128, (m + 1) * 128)
                    nc.tensor.matmul(psh[:, m, :], w["wvofc1"][:, 0, ms],
                                     enct[:, 0, :], start=True, stop=False)
                    nc.tensor.matmul(psh[:, m, :], w["wvofc1"][:, 1, ms],
                                     enct[:, 1, :], start=False, stop=False)
                    nc.tensor.matmul(psh[:, m, :], w["wofc1"][:, 0, ms],
                                     tmp[:, 0, :], start=False, stop=False)
                    nc.tensor.matmul(psh[:, m, :], w["wofc1"][:, 1, ms],
                                     tmp[:, 1, :], start=False, stop=True)
                h1 = wp.tile([128, 2, T], BF, tag="h1", bufs=3, name="h1")
                relu_evac(h1, psh, None, "bh1b")
                H1[i] = h1

            def b_s5(i):
                h1 = H1.pop(i)
                pso = pp.tile([128, T], F32, tag="ps_small", bufs=2, name="pso")
                for k in range(2):
                    nc.tensor.matmul(pso[:NCLS, :], w["wc2"][:, k, :], h1[:, k, :],
                                     start=(k == 0), stop=(k == 1))
                osb = wp.tile([NCLS, T], F32, tag="osb", bufs=3, name="osb")
                if zero_bias:
                    nc.vector.tensor_copy(osb[:], pso[:NCLS, :])
                else:
                    nc.vector.tensor_scalar_add(osb[:], pso[:NCLS, :],
                                                w["bc2"][:NCLS, :])
                nc.gpsimd.dma_start(dram["outb"][i], osb[:])

            if nbt:
                for i0 in range(min(3, nbt)):
                    b_s1(i0)
                b_s2(0)
                for i in range(nbt + 2):
                    if i < nbt:
                        b_s3a(i)
                    if i + 1 < nbt:
                        b_s2(i + 1)
                    if i < nbt:
                        b_s3b(i)
                    if 0 <= i - 1 < nbt:
                        b_s4(i - 1)
                    if 0 <= i - 2 < nbt:
                        b_s5(i - 2)
                    if i + 3 < nbt:
                        b_s1(i + 3)

            # ================= single-modality pipelines =================
            # out = relu(enc @ Wpc1 + bh1) @ Wc2 + bc2, enc = relu(x@We+be)
            def only_pipe(x_dram, out_dram, ntiles, wenc, benc, wproj, bproj):
                XO, ENO, HO = {}, {}, {}

                def o_s1(j):
                    xo = wp.tile([128, 4, T], BF, tag="xi", bufs=4, name="xo")
                    nc.sync.dma_start(xo[:], x_dram[j])
                    XO[j] = xo

                def o_s2(j):
                    xo = XO.pop(j)
                    pe = pp.tile([128, 2, T], F32, tag="ps_big", bufs=3, name="peo")
                    for m in range(2):
                        for k in range(4):
                            nc.tensor.matmul(
                                pe[:, m, :], w[wenc][:, k, m * 128 : (m + 1) * 128],
                                xo[:, k, :], start=(k == 0), stop=(k == 3),
                            )
                    eno = wp.tile([128, 2, T], BF, tag="enct", bufs=3, name="eno")
                    relu_evac(eno, pe, None, benc)
                    ENO[j] = eno

                def o_s3(j):
                    eno = ENO.pop(j)
                    psh = pp.tile([128, 2, T], F32, tag="ps_big", bufs=3, name="psho")
                    for m in range(2):
                        ms = slice(m * 128, (m + 1) * 128)
                        for k in range(2):
                            nc.tensor.matmul(psh[:, m, :], w[wproj][:, k, ms],
                                             eno[:, k, :], start=(k == 0),
                                             stop=(k == 1))
                    h1 = wp.tile([128, 2, T], BF, tag="h1", bufs=3, name="h1o")
                    if zero_bias:
                        nc.vector.tensor_scalar_max(h1[:, :, :], psh[:, :, :], 0.0)
                    else:
                        for m in range(2):
                            nc.scalar.activation(
                                h1[:, m, :], psh[:, m, :], AF.Relu,
                                bias=w[bproj][:, m : m + 1],
                            )
                    HO[j] = h1

                def o_s4(j):
                    h1 = HO.pop(j)
                    pso = pp.tile([128, T], F32, tag="ps_small", bufs=2, name="psoo")
                    for k in range(2):
                        nc.tensor.matmul(pso[:NCLS, :], w["wc2"][:, k, :],
                                         h1[:, k, :], start=(k == 0), stop=(k == 1))
                    osb = wp.tile([NCLS, T], F32, tag="osb", bufs=3, name="osbo")
                    if zero_bias:
                        nc.vector.tensor_copy(osb[:], pso[:NCLS, :])
                    else:
                        nc.vector.tensor_scalar_add(osb[:], pso[:NCLS, :],
                                                    w["bc2"][:NCLS, :])
                    nc.gpsimd.dma_start(out_dram[j], osb[:])

                for j0 in range(min(3, ntiles)):
                    o_s1(j0)
                o_s2(0)
                for j in range(ntiles + 1):
                    if j + 1 < ntiles:
                        o_s2(j + 1)
                    if j < ntiles:
                        o_s3(j)
                    if 0 <= j - 1 < ntiles:
                        o_s4(j - 1)
                    if j + 3 < ntiles:
                        o_s1(j + 3)

            if nit:
                only_pipe(dram["xi_img"], dram["outi"], nit, "wie", "bie", "wipc1", "bh1i")
            if ntt:
                only_pipe(dram["xt_txt"], dram["outt"], ntt, "wte", "bte", "wtpc1", "bh1t")

    nc.compile()
    return nc


def _prep_weights(inp):
    """Host-side weight prep: fold/merge/transpose into the device layouts."""
    f32 = np.float32
    q = (inp["fusion_query"].reshape(1, H).astype(f32) @ inp["Wq"] + inp["bq"]).reshape(
        NH, HD
    )
    wkeff = np.zeros((H, NH), f32)
    for h in range(NH):
        wkeff[:, h] = inp["Wk"][:, h * HD : (h + 1) * HD] @ q[h] / np.sqrt(HD)
    wof = inp["Wo"].astype(f32) @ inp["W_fp"]
    bof = inp["bo"].astype(f32) @ inp["W_fp"] + inp["b_fp"]
    wofc1 = wof @ inp["Wc1"]
    wvofc1 = inp["Wv"].astype(f32) @ wofc1
    bh1b = inp["bv"].astype(f32) @ wofc1 + bof @ inp["Wc1"] + inp["bc1"]
    wipc1 = inp["W_ip"].astype(f32) @ inp["Wc1"]
    bh1i = inp["b_ip"].astype(f32) @ inp["Wc1"] + inp["bc1"]
    wtpc1 = inp["W_tp"].astype(f32) @ inp["Wc1"]
    bh1t = inp["b_tp"].astype(f32) @ inp["Wc1"] + inp["bc1"]
    emat = np.zeros((NH, H), f32)
    for h in range(NH):
        emat[h, h * HD : (h + 1) * HD] = 1.0

    def ktile(a, kt):  # [K, M] -> [128, kt, M]
        return np.ascontiguousarray(
            a.reshape(kt, 128, a.shape[1]).transpose(1, 0, 2)
        )

    bf = ml_dtypes.bfloat16
    out = {
        "wie": ktile(inp["W_ie"], 4).astype(bf),
        "wte": ktile(inp["W_te"], 4).astype(bf),
        "wkeff": ktile(wkeff, 2).astype(bf),
        "emat": np.ascontiguousarray(emat.reshape(NH, 2, 128)).astype(bf),
        "wv": ktile(inp["Wv"].astype(f32), 2).astype(bf),
        "wvofc1": ktile(wvofc1, 2).astype(bf),
        "wofc1": ktile(wofc1, 2).astype(bf),
        "wipc1": ktile(wipc1, 2).astype(bf),
        "wtpc1": ktile(wtpc1, 2).astype(bf),
        "wc2": ktile(inp["Wc2"].astype(f32), 2).astype(bf),
        "bie": np.ascontiguousarray(inp["b_ie"].reshape(2, 128).T).astype(f32),
        "bte": np.ascontiguousarray(inp["b_te"].reshape(2, 128).T).astype(f32),
        "bh1b": np.ascontiguousarray(bh1b.reshape(2, 128).T).astype(f32),
        "bh1i": np.ascontiguousarray(bh1i.reshape(2, 128).T).astype(f32),
        "bh1t": np.ascontiguousarray(bh1t.reshape(2, 128).T).astype(f32),
        "bc2": np.ascontiguousarray(
            np.pad(inp["bc2"].astype(f32), (0, 128 - NCLS)).reshape(128, 1)
        ),
    }
    return out


def _split_pad(idx):
    """Split index array across cores evenly; pad each core's slice to a
    multiple of T with -1. Returns list of per-core padded index arrays
    (all the same length)."""
    per = [idx[c::NCORES] for c in range(NCORES)]
    n = max(len(p) for p in per)
    npad = ((n + T - 1) // T) * T if n else 0
    out = []
    for p in per:
        a = np.full(npad, -1, dtype=np.int64)
        a[: len(p)] = p
        out.append(a)
    return out


def _gather_tiles(x_bf, idx):
    """Rows idx of x (with -1 -> zero row), as tile-major blocks
    [ntiles, 128, 4, T] bf16: block[j, p, k, c] = x[idx[j*T+c], k*128+p]."""
    n = len(idx)
    d = x_bf.shape[1]
    g = np.zeros((n, d), dtype=x_bf.dtype)
    valid = idx >= 0
    g[valid] = x_bf[idx[valid]]
    return np.ascontiguousarray(g.reshape(n // T, T, 4, 128).transpose(0, 3, 2, 1))


def _ntff_hook():
    """Build the (output_dir, device_ids) -> contextmanager NTFF profile
    hook directly via ctypes on the axon PJRT .so (the image's antenv lacks
    axon_hooks, so the boot-time registration was skipped)."""
    import ctypes
    import contextlib

    so_path = "/opt/axon/libaxon_pjrt.so"
    lib = ctypes.CDLL(so_path)
    if not hasattr(lib, "axon_start_nrt_profile"):
        return None
    lib.axon_start_nrt_profile.argtypes = [
        ctypes.POINTER(ctypes.c_int64),
        ctypes.c_size_t,
    ]
    lib.axon_start_nrt_profile.restype = ctypes.c_int64
    lib.axon_stop_nrt_profile.argtypes = [ctypes.c_char_p]
    lib.axon_stop_nrt_profile.restype = ctypes.c_int64

    @contextlib.contextmanager
    def _hook(output_dir, device_ids):
        import jax

        jax.devices()
        if device_ids:
            ids = (ctypes.c_int64 * len(device_ids))(*device_ids)
            rc = lib.axon_start_nrt_profile(ids, len(device_ids))
        else:
            rc = lib.axon_start_nrt_profile(None, 0)
        if rc != 0:
            raise RuntimeError(f"axon_start_nrt_profile rc={rc}")
        try:
            yield
        finally:
            n = lib.axon_stop_nrt_profile(str(output_dir).encode())
            print(f"profile: {n} file(s) written to {output_dir}", file=sys.stderr)

    return _hook


def _profiled_run(nc, in_maps):
    """Run via PJRT with NTFF profiling; parse exec_time_ns from the trace."""
    import tempfile
    import glob as _glob

    from concourse import bass2jax
    from concourse._compat import FishPath
    import gauge.profiler

    hook = _ntff_hook()
    tmpdir = tempfile.mkdtemp(prefix="aecf_prof_")
    if hook is None:
        results = bass2jax.run_bass_via_pjrt(nc, in_maps, n_cores=NCORES)
        return results, None, None
    with hook(tmpdir, [0]):
        results = bass2jax.run_bass_via_pjrt(nc, in_maps, n_cores=NCORES)
    ntffs = _glob.glob(os.path.join(tmpdir, "*_body*.ntff"))
    if not ntffs:
        print(f"no NTFFs in {tmpdir}: {sorted(os.listdir(tmpdir))}", file=sys.stderr)
        return results, None, None
    prof = gauge.profiler.Profile(
        profile_path=FishPath(tmpdir),
        kernel_dev_mode=True,
        profile_on_exit=False,
        bass_kernel=nc.m,
        offline_processing=True,
        fname="*_body*",
        metadata={},
    )
    try:
        pres = prof.to_perfetto(model_index=(0,))
        exec_ns = pres[0].exec_time_ns if pres else None
        pjson = prof.json_path(0).path if pres else None
    except Exception as e:
        print(f"profile parse failed: {e}", file=sys.stderr)
        return results, None, None
    return results, exec_ns, pjson


def kernel(**inputs):
    global LAST_EXEC_NS, LAST_PROFILE
    img = np.asarray(inputs["image_features"], dtype=np.float32)
    txt = np.asarray(inputs["text_features"], dtype=np.float32)

    pres_i = np.linalg.norm(img, axis=1) > 1e-6
    pres_t = np.linalg.norm(txt, axis=1) > 1e-6
    both = pres_i & pres_t
    oi = pres_i & ~pres_t
    ot = ~pres_i & pres_t
    none = ~pres_i & ~pres_t

    idx_b = _split_pad(np.nonzero(both)[0])
    idx_i = _split_pad(np.nonzero(oi)[0])
    idx_t = _split_pad(np.nonzero(ot)[0])
    nb, ni, nt = len(idx_b[0]), len(idx_i[0]), len(idx_t[0])

    bias_names = ("b_ie", "b_te", "bv", "bo", "b_fp", "b_ip", "b_tp", "bc1", "bc2")
    zero_bias = all(not np.any(np.asarray(inputs[n])) for n in bias_names)
    key = (nb, ni, nt, zero_bias)
    if key not in _GRAPH_CACHE:
        _GRAPH_CACHE[key] = _build_graph(nb, ni, nt, zero_bias)
    nc = _GRAPH_CACHE[key]

    wmap = _prep_weights(inputs)
    bf = ml_dtypes.bfloat16
    img_bf = img.astype(bf)
    txt_bf = txt.astype(bf)

    in_maps = []
    for c in range(NCORES):
        m = dict(wmap)
        if nb:
            m["xb_img"] = _gather_tiles(img_bf, idx_b[c])
            m["xb_txt"] = _gather_tiles(txt_bf, idx_b[c])
        if ni:
            m["xi_img"] = _gather_tiles(img_bf, idx_i[c])
        if nt:
            m["xt_txt"] = _gather_tiles(txt_bf, idx_t[c])
        in_maps.append(m)

    trace = bool(int(os.environ.get("KERNEL_PROFILE", "0")))
    if trace:
        results, exec_ns, prof_json = _profiled_run(nc, in_maps)
        LAST_EXEC_NS = exec_ns
        LAST_PROFILE = prof_json

        class _R:
            pass

        res = _R()
        res.results = results
    else:
        res = run_bass_kernel_spmd(nc, in_maps, core_ids=list(range(NCORES)))
        LAST_EXEC_NS = None
        LAST_PROFILE = None

    logits = np.empty((img.shape[0], NCLS), dtype=np.float32)
    for c in range(NCORES):
        r = res.results[c]
        for name, idx in (("outb", idx_b[c]), ("outi", idx_i[c]), ("outt", idx_t[c])):
            if name in r:
                valid = idx >= 0
                flat = np.ascontiguousarray(r[name].transpose(0, 2, 1)).reshape(
                    -1, NCLS
                )
                logits[idx[valid]] = flat[valid]

    if none.any():
        # reference: fused = 0 -> logits = relu(bc1) @ Wc2 + bc2 (constant)
        row = (
            np.maximum(inputs["bc1"].astype(np.float32), 0.0) @ inputs["Wc2"]
            + inputs["bc2"]
        )
        logits[none] = row
    return logits


# revision 16
# speedup vs baseline: 1.5026x; 1.0356x over previous
"""AECF multimodal fusion kernel for 8 TRN2 NeuronCores.

Strategy:
  - Host-side routing (part of sharding): rows are sorted into three branch
    groups (both modalities present / only-image / only-text) using the same
    norm>1e-6 predicate as the reference. Each group is dealt evenly across
    the 8 cores and padded to a tile multiple; the NEFF is compiled with the
    actual per-core group sizes (compile happens inside kernel(), after the
    inputs are known), so the graph is static and identical on all cores.
  - Tile-major feature-major layout: inputs are shipped as [ntile, 128, 4, T]
    bf16 blocks (4KB contiguous per partition per tile), outputs leave as
    [ntile, 80, T] f32 blocks.
  - bf16 storage/compute, f32 PSUM accumulation.
  - Algebra: scores only enter through a 2-way softmax, so
    attn_img = sigmoid(s_img - s_txt) with s = enc @ Wk_eff,
    Wk_eff[:, h] = Wk[:, 64h:64h+64] @ q[h] / 8 (bk cancels in the diff).
    There is no nonlinearity between the post-attention projection and the
    classifier's first layer, so Wo, W_fp, Wc1 merge into Wofc1 [256,256]:
      h1pre = pooled @ Wofc1 + bh1
            = enc_t @ (Wv@Wofc1) + (A * v_d) @ Wofc1 + bh1
    with v_d = (enc_i - enc_t) @ Wv and A = head-broadcast(attn) (PE matmul
    with a 0/1 expander).  v_txt is never materialized.  Only-branches use
    Wipc1 = W_ip@Wc1 / Wtpc1 = W_tp@Wc1.
  - Depth-3 software pipeline per both-tile, ordered so the PE queue (the
    bottleneck engine) never waits on ACT/DVE round trips; PSUM tiles span
    2 banks so evacuations/relus are single ops over [128, 1024].
"""

import os
import sys

if "/opt/trn_rl_repo" not in sys.path:
    sys.path.insert(0, "/opt/trn_rl_repo")

import numpy as np
import ml_dtypes

import concourse.bass as bass
import concourse.bacc as bacc
import concourse.tile as tile
from concourse import mybir
from concourse.bass_utils import run_bass_kernel_spmd

BF = mybir.dt.bfloat16
F32 = mybir.dt.float32
AF = mybir.ActivationFunctionType
OP = mybir.AluOpType

H = 256
ID = 512
TD = 512
NCLS = 80
NH = 4
HD = 64
B = 131072
NCORES = 8
T = 512  # batch-tile (free-dim) size; one psum bank of f32 per 128-chunk

LAST_EXEC_NS = None
LAST_PROFILE = None

_GRAPH_CACHE = {}


def _build_graph(nb, ni, nt, zero_bias):
    """Build the SPMD graph for per-core group column counts nb/ni/nt
    (each a multiple of T, possibly 0)."""
    nc = bacc.Bacc()
    nbt, nit, ntt = nb // T, ni // T, nt // T

    # ---- DRAM I/O ----
    dram = {}
    if nbt:
        dram["xb"] = nc.dram_tensor("xb", [nbt, 128, 8, T], BF, kind="ExternalInput")
        dram["outb"] = nc.dram_tensor("outb", [nbt, NCLS, T], F32, kind="ExternalOutput")
    if nit:
        dram["xi_img"] = nc.dram_tensor("xi_img", [nit, 128, 4, T], BF, kind="ExternalInput")
        dram["outi"] = nc.dram_tensor("outi", [nit, NCLS, T], F32, kind="ExternalOutput")
    if ntt:
        dram["xt_txt"] = nc.dram_tensor("xt_txt", [ntt, 128, 4, T], BF, kind="ExternalInput")
        dram["outt"] = nc.dram_tensor("outt", [ntt, NCLS, T], F32, kind="ExternalOutput")

    wspec = {
        "wie": ([128, 4, H], BF),
        "wte": ([128, 4, H], BF),
        "wkeff": ([128, 2, NH], BF),
        "emat": ([NH, 2, 128], BF),
        "wv": ([128, 2, H], BF),
        "wvofc1": ([128, 2, H], BF),
        "wofc1": ([128, 2, H], BF),
        "wipc1": ([128, 2, H], BF),
        "wtpc1": ([128, 2, H], BF),
        "wc2": ([128, 2, NCLS], BF),
        "bie": ([128, 2], F32),
        "bte": ([128, 2], F32),
        "bh1b": ([128, 2], F32),
        "bh1i": ([128, 2], F32),
        "bh1t": ([128, 2], F32),
        "bc2": ([128, 1], F32),
    }
    for name, (shape, dt) in wspec.items():
        dram[name] = nc.dram_tensor(name, shape, dt, kind="ExternalInput")

    with tile.TileContext(nc) as tc:
        with (
            tc.tile_pool(name="wpool", bufs=1) as wpool,
            tc.tile_pool(name="work", bufs=2) as wp,
            tc.tile_pool(name="psum", bufs=1, space="PSUM") as pp,
        ):
            w = {}
            for name, (shape, dt) in wspec.items():
                w[name] = wpool.tile(shape, dt, tag=name, name=name)
                # encoder weights ride the sync queue ahead of the input
                # tiles; everything else streams on gpsimd
                eng = nc.sync if name in ("wie", "wte") else nc.gpsimd
                eng.dma_start(w[name][:], dram[name][:])

            def relu_evac(dst, ps, bias, tag2):
                """psum [128,2,T] -> sbuf bf16 with relu (+bias per m-half)."""
                if zero_bias:
                    nc.scalar.activation(dst[:, :, :], ps[:, :, :], AF.Relu)
                else:
                    for m in range(2):
                        nc.scalar.activation(
                            dst[:, m, :], ps[:, m, :], AF.Relu,
                            bias=w[tag2][:, m : m + 1],
                        )

            # ================= both-modality pipeline =================
            # Stages per tile i:
            #   S1: input DMAs      S2: encoders (PE 16) + relus + denc
            #   S3a: wkeff (PE 2) + sigmoid     S3b: vd (PE 4) + emat (PE 2)
            #        + vd evac + tmp = A*vd
            #   S4: h1pre (PE 8: enc_t@Wvofc1 + tmp@Wofc1) + h1 relu
            #   S5: wc2 (PE 2) + out evac + out DMA
            # Iteration i emits: S3a(i), S2(i+1), S3b(i), S4(i-1), S5(i-2),
            # S1(i+2) so the in-order PE queue never waits on ACT/DVE.
            X, ENC, DNC, ATT, TMP, H1 = {}, {}, {}, {}, {}, {}

            def b_s1(i):
                xb = wp.tile([128, 8, T], BF, tag="xi", bufs=4, name="xb")
                nc.sync.dma_start(xb[:], dram["xb"][i])
                X[i] = xb

            def b_s2(i):
                xb = X.pop(i)
                xi = xb[:, 0:4, :]
                xt = xb[:, 4:8, :]
                pei = pp.tile([128, 2, T], F32, tag="ps_big", bufs=3, name="pei")
                for m in range(2):
                    for k in range(4):
                        nc.tensor.matmul(
                            pei[:, m, :], w["wie"][:, k, m * 128 : (m + 1) * 128],
                            xi[:, k, :], start=(k == 0), stop=(k == 3),
                        )
                enci = wp.tile([128, 2, T], BF, tag="enci", bufs=2, name="enci")
                relu_evac(enci, pei, None, "bie")
                pet = pp.tile([128, 2, T], F32, tag="ps_big", bufs=3, name="pet")
                for m in range(2):
                    for k in range(4):
                        nc.tensor.matmul(
                            pet[:, m, :], w["wte"][:, k, m * 128 : (m + 1) * 128],
                            xt[:, k, :], start=(k == 0), stop=(k == 3),
                        )
                enct = wp.tile([128, 2, T], BF, tag="enct", bufs=3, name="enct")
                relu_evac(enct, pet, None, "bte")
                denc = wp.tile([128, 2, T], BF, tag="denc", bufs=3, name="denc")
                nc.vector.tensor_tensor(denc[:, :, :], enci[:, :, :], enct[:, :, :],
                                        op=OP.subtract)
                ENC[i] = enct
                DNC[i] = denc

            def b_s3a(i):
                d = DNC[i]
                pss = pp.tile([128, T], F32, tag="ps_small", bufs=2, name="pss")
                for k in range(2):
                    nc.tensor.matmul(pss[:NH, :], w["wkeff"][:, k, :], d[:, k, :],
                                     start=(k == 0), stop=(k == 1))
                att = wp.tile([NH, T], BF, tag="att", bufs=3, name="att")
                nc.scalar.activation(att[:], pss[:NH, :], AF.Sigmoid)
                ATT[i] = att

            def b_s3b(i):
                d = DNC.pop(i)
                att = ATT.pop(i)
                psv = pp.tile([128, 2, T], F32, tag="ps_big", bufs=3, name="psv")
                for m in range(2):
                    for k in range(2):
                        nc.tensor.matmul(
                            psv[:, m, :], w["wv"][:, k, m * 128 : (m + 1) * 128],
                            d[:, k, :], start=(k == 0), stop=(k == 1),
                        )
                psa = pp.tile([128, 2, T], F32, tag="ps_big", bufs=3, name="psa")
                for m in range(2):
                    nc.tensor.matmul(psa[:, m, :], w["emat"][:, m, :], att[:],
                                     start=True, stop=True)
                vd = wp.tile([128, 2, T], BF, tag="vd", bufs=2, name="vd")
                nc.vector.tensor_copy(vd[:, :, :], psv[:, :, :])
                tmp = wp.tile([128, 2, T], BF, tag="tmp", bufs=3, name="tmp")
                nc.vector.tensor_tensor(tmp[:, :, :], psa[:, :, :], vd[:, :, :],
                                        op=OP.mult)
                TMP[i] = tmp

            def b_s4(i):
                enct = ENC.pop(i)
                tmp = TMP.pop(i)
                psh = pp.tile([128, 2, T], F32, tag="ps_big", bufs=3, name="psh")
                for m in range(2):
                    ms = slice(m * 128, (m + 1) * 128)
                    nc.tensor.matmul(psh[:, m, :], w["wvofc1"][:, 0, ms],
                                     enct[:, 0, :], start=True, stop=False)
                    nc.tensor.matmul(psh[:, m, :], w["wvofc1"][:, 1, ms],
                                     enct[:, 1, :], start=False, stop=False)
                    nc.tensor.matmul(psh[:, m, :], w["wofc1"][:, 0, ms],
                                     tmp[:, 0, :], start=False, stop=False)
                    nc.tensor.matmul(psh[:, m, :], w["wofc1"][:, 1, ms],
                                     tmp[:, 1, :], start=False, stop=True)
                h1 = wp.tile([128, 2, T], BF, tag="h1", bufs=3, name="h1")
                relu_evac(h1, psh, None, "bh1b")
                H1[i] = h1

            def b_s5(i):
                h1 = H1.pop(i)
                pso = pp.tile([128, T], F32, tag="ps_small", bufs=2, name="pso")
                for k in range(2):
                    nc.tensor.matmul(pso[:NCLS, :], w["wc2"][:, k, :], h1[:, k, :],
                                     start=(k == 0), stop=(k == 1))
                osb = wp.tile([NCLS, T], F32, tag="osb", bufs=3, name="osb")
                if zero_bias:
                    nc.vector.tensor_copy(osb[:], pso[:NCLS, :])
                else:
                    nc.vector.tensor_scalar_add(osb[:], pso[:NCLS, :],
                                                w["bc2"][:NCLS, :])
                nc.gpsimd.dma_start(dram["outb"][i], osb[:])

            if nbt:
                for i0 in range(min(3, nbt)):
                    b_s1(i0)
                b_s2(0)
                for i in range(nbt + 2):
                    if i < nbt:
                        b_s3a(i)
                    if i + 1 < nbt:
                        b_s2(i + 1)
                    if i < nbt:
                        b_s3b(i)
                    if 0 <= i - 1 < nbt:
                        b_s4(i - 1)
                    if 0 <= i - 2 < nbt:
                        b_s5(i - 2)
                    if i + 3 < nbt:
                        b_s1(i + 3)

            # ================= single-modality pipelines =================
            # out = relu(enc @ Wpc1 + bh1) @ Wc2 + bc2, enc = relu(x@We+be)
            def only_pipe(x_dram, out_dram, ntiles, wenc, benc, wproj, bproj, out_eng):
                XO, ENO, HO = {}, {}, {}

                def o_s1(j):
                    xo = wp.tile([128, 4, T], BF, tag="xi", bufs=4, name="xo")
                    nc.sync.dma_start(xo[:], x_dram[j])
                    XO[j] = xo

                def o_s2(j):
                    xo = XO.pop(j)
                    pe = pp.tile([128, 2, T], F32, tag="ps_big", bufs=3, name="peo")
                    for m in range(2):
                        for k in range(4):
                            nc.tensor.matmul(
                                pe[:, m, :], w[wenc][:, k, m * 128 : (m + 1) * 128],
                                xo[:, k, :], start=(k == 0), stop=(k == 3),
                            )
                    eno = wp.tile([128, 2, T], BF, tag="enct", bufs=3, name="eno")
                    relu_evac(eno, pe, None, benc)
                    ENO[j] = eno

                def o_s3(j):
                    eno = ENO.pop(j)
                    psh = pp.tile([128, 2, T], F32, tag="ps_big", bufs=3, name="psho")
                    for m in range(2):
                        ms = slice(m * 128, (m + 1) * 128)
                        for k in range(2):
                            nc.tensor.matmul(psh[:, m, :], w[wproj][:, k, ms],
                                             eno[:, k, :], start=(k == 0),
                                             stop=(k == 1))
                    h1 = wp.tile([128, 2, T], BF, tag="h1", bufs=3, name="h1o")
                    if zero_bias:
                        nc.vector.tensor_scalar_max(h1[:, :, :], psh[:, :, :], 0.0)
                    else:
                        for m in range(2):
                            nc.scalar.activation(
                                h1[:, m, :], psh[:, m, :], AF.Relu,
                                bias=w[bproj][:, m : m + 1],
                            )
                    HO[j] = h1

                def o_s4(j):
                    h1 = HO.pop(j)
                    pso = pp.tile([128, T], F32, tag="ps_small", bufs=2, name="psoo")
                    for k in range(2):
                        nc.tensor.matmul(pso[:NCLS, :], w["wc2"][:, k, :],
                                         h1[:, k, :], start=(k == 0), stop=(k == 1))
                    osb = wp.tile([NCLS, T], F32, tag="osb", bufs=3, name="osbo")
                    if zero_bias:
                        nc.vector.tensor_copy(osb[:], pso[:NCLS, :])
                    else:
                        nc.vector.tensor_scalar_add(osb[:], pso[:NCLS, :],
                                                    w["bc2"][:NCLS, :])
                    out_eng.dma_start(out_dram[j], osb[:])

                for j0 in range(min(3, ntiles)):
                    o_s1(j0)
                o_s2(0)
                for j in range(ntiles + 1):
                    if j + 1 < ntiles:
                        o_s2(j + 1)
                    if j < ntiles:
                        o_s3(j)
                    if 0 <= j - 1 < ntiles:
                        o_s4(j - 1)
                    if j + 3 < ntiles:
                        o_s1(j + 3)

            if nit:
                only_pipe(dram["xi_img"], dram["outi"], nit, "wie", "bie", "wipc1",
                          "bh1i", nc.gpsimd)
            if ntt:
                only_pipe(dram["xt_txt"], dram["outt"], ntt, "wte", "bte", "wtpc1",
                          "bh1t", nc.scalar)

    nc.compile()
    return nc


def _prep_weights(inp):
    """Host-side weight prep: fold/merge/transpose into the device layouts."""
    f32 = np.float32
    q = (inp["fusion_query"].reshape(1, H).astype(f32) @ inp["Wq"] + inp["bq"]).reshape(
        NH, HD
    )
    wkeff = np.zeros((H, NH), f32)
    for h in range(NH):
        wkeff[:, h] = inp["Wk"][:, h * HD : (h + 1) * HD] @ q[h] / np.sqrt(HD)
    wof = inp["Wo"].astype(f32) @ inp["W_fp"]
    bof = inp["bo"].astype(f32) @ inp["W_fp"] + inp["b_fp"]
    wofc1 = wof @ inp["Wc1"]
    wvofc1 = inp["Wv"].astype(f32) @ wofc1
    bh1b = inp["bv"].astype(f32) @ wofc1 + bof @ inp["Wc1"] + inp["bc1"]
    wipc1 = inp["W_ip"].astype(f32) @ inp["Wc1"]
    bh1i = inp["b_ip"].astype(f32) @ inp["Wc1"] + inp["bc1"]
    wtpc1 = inp["W_tp"].astype(f32) @ inp["Wc1"]
    bh1t = inp["b_tp"].astype(f32) @ inp["Wc1"] + inp["bc1"]
    emat = np.zeros((NH, H), f32)
    for h in range(NH):
        emat[h, h * HD : (h + 1) * HD] = 1.0

    def ktile(a, kt):  # [K, M] -> [128, kt, M]
        return np.ascontiguousarray(
            a.reshape(kt, 128, a.shape[1]).transpose(1, 0, 2)
        )

    bf = ml_dtypes.bfloat16
    out = {
        "wie": ktile(inp["W_ie"], 4).astype(bf),
        "wte": ktile(inp["W_te"], 4).astype(bf),
        "wkeff": ktile(wkeff, 2).astype(bf),
        "emat": np.ascontiguousarray(emat.reshape(NH, 2, 128)).astype(bf),
        "wv": ktile(inp["Wv"].astype(f32), 2).astype(bf),
        "wvofc1": ktile(wvofc1, 2).astype(bf),
        "wofc1": ktile(wofc1, 2).astype(bf),
        "wipc1": ktile(wipc1, 2).astype(bf),
        "wtpc1": ktile(wtpc1, 2).astype(bf),
        "wc2": ktile(inp["Wc2"].astype(f32), 2).astype(bf),
        "bie": np.ascontiguousarray(inp["b_ie"].reshape(2, 128).T).astype(f32),
        "bte": np.ascontiguousarray(inp["b_te"].reshape(2, 128).T).astype(f32),
        "bh1b": np.ascontiguousarray(bh1b.reshape(2, 128).T).astype(f32),
        "bh1i": np.ascontiguousarray(bh1i.reshape(2, 128).T).astype(f32),
        "bh1t": np.ascontiguousarray(bh1t.reshape(2, 128).T).astype(f32),
        "bc2": np.ascontiguousarray(
            np.pad(inp["bc2"].astype(f32), (0, 128 - NCLS)).reshape(128, 1)
        ),
    }
    return out


def _split_pad(idx):
    """Split index array across cores evenly; pad each core's slice to a
    multiple of T with -1. Returns list of per-core padded index arrays
    (all the same length)."""
    per = [idx[c::NCORES] for c in range(NCORES)]
    n = max(len(p) for p in per)
    npad = ((n + T - 1) // T) * T if n else 0
    out = []
    for p in per:
        a = np.full(npad, -1, dtype=np.int64)
        a[: len(p)] = p
        out.append(a)
    return out


def _gather_tiles(x_bf, idx):
    """Rows idx of x (with -1 -> zero row), as tile-major blocks
    [ntiles, 128, 4, T] bf16: block[j, p, k, c] = x[idx[j*T+c], k*128+p]."""
    n = len(idx)
    d = x_bf.shape[1]
    g = np.zeros((n, d), dtype=x_bf.dtype)
    valid = idx >= 0
    g[valid] = x_bf[idx[valid]]
    return np.ascontiguousarray(g.reshape(n // T, T, 4, 128).transpose(0, 3, 2, 1))


def _ntff_hook():
    """Build the (output_dir, device_ids) -> contextmanager NTFF profile
    hook directly via ctypes on the axon PJRT .so (the image's antenv lacks
    axon_hooks, so the boot-time registration was skipped)."""
    import ctypes
    import contextlib

    so_path = "/opt/axon/libaxon_pjrt.so"
    lib = ctypes.CDLL(so_path)
    if not hasattr(lib, "axon_start_nrt_profile"):
        return None
    lib.axon_start_nrt_profile.argtypes = [
        ctypes.POINTER(ctypes.c_int64),
        ctypes.c_size_t,
    ]
    lib.axon_start_nrt_profile.restype = ctypes.c_int64
    lib.axon_stop_nrt_profile.argtypes = [ctypes.c_char_p]
    lib.axon_stop_nrt_profile.restype = ctypes.c_int64

    @contextlib.contextmanager
    def _hook(output_dir, device_ids):
        import jax

        jax.devices()
        if device_ids:
            ids = (ctypes.c_int64 * len(device_ids))(*device_ids)
            rc = lib.axon_start_nrt_profile(ids, len(device_ids))
        else:
            rc = lib.axon_start_nrt_profile(None, 0)
        if rc != 0:
            raise RuntimeError(f"axon_start_nrt_profile rc={rc}")
        try:
            yield
        finally:
            n = lib.axon_stop_nrt_profile(str(output_dir).encode())
            print(f"profile: {n} file(s) written to {output_dir}", file=sys.stderr)

    return _hook


def _profiled_run(nc, in_maps):
    """Run via PJRT with NTFF profiling; parse exec_time_ns from the trace."""
    import tempfile
    import glob as _glob

    from concourse import bass2jax
    from concourse._compat import FishPath
    import gauge.profiler

    hook = _ntff_hook()
    tmpdir = tempfile.mkdtemp(prefix="aecf_prof_")
    if hook is None:
        results = bass2jax.run_bass_via_pjrt(nc, in_maps, n_cores=NCORES)
        return results, None, None
    with hook(tmpdir, [0]):
        results = bass2jax.run_bass_via_pjrt(nc, in_maps, n_cores=NCORES)
    ntffs = _glob.glob(os.path.join(tmpdir, "*_body*.ntff"))
    if not ntffs:
        print(f"no NTFFs in {tmpdir}: {sorted(os.listdir(tmpdir))}", file=sys.stderr)
        return results, None, None
    prof = gauge.profiler.Profile(
        profile_path=FishPath(tmpdir),
        kernel_dev_mode=True,
        profile_on_exit=False,
        bass_kernel=nc.m,
        offline_processing=True,
        fname="*_body*",
        metadata={},
    )
    try:
        pres = prof.to_perfetto(model_index=(0,))
        exec_ns = pres[0].exec_time_ns if pres else None
        pjson = prof.json_path(0).path if pres else None
    except Exception as e:
        print(f"profile parse failed: {e}", file=sys.stderr)
        return results, None, None
    return results, exec_ns, pjson


def kernel(**inputs):
    global LAST_EXEC_NS, LAST_PROFILE
    img = np.asarray(inputs["image_features"], dtype=np.float32)
    txt = np.asarray(inputs["text_features"], dtype=np.float32)

    pres_i = np.linalg.norm(img, axis=1) > 1e-6
    pres_t = np.linalg.norm(txt, axis=1) > 1e-6
    both = pres_i & pres_t
    oi = pres_i & ~pres_t
    ot = ~pres_i & pres_t
    none = ~pres_i & ~pres_t

    idx_b = _split_pad(np.nonzero(both)[0])
    idx_i = _split_pad(np.nonzero(oi)[0])
    idx_t = _split_pad(np.nonzero(ot)[0])
    nb, ni, nt = len(idx_b[0]), len(idx_i[0]), len(idx_t[0])

    bias_names = ("b_ie", "b_te", "bv", "bo", "b_fp", "b_ip", "b_tp", "bc1", "bc2")
    zero_bias = all(not np.any(np.asarray(inputs[n])) for n in bias_names)
    key = (nb, ni, nt, zero_bias)
    if key not in _GRAPH_CACHE:
        _GRAPH_CACHE[key] = _build_graph(nb, ni, nt, zero_bias)
    nc = _GRAPH_CACHE[key]

    wmap = _prep_weights(inputs)
    bf = ml_dtypes.bfloat16
    img_bf = img.astype(bf)
    txt_bf = txt.astype(bf)

    in_maps = []
    for c in range(NCORES):
        m = dict(wmap)
        if nb:
            m["xb"] = np.concatenate(
                [_gather_tiles(img_bf, idx_b[c]), _gather_tiles(txt_bf, idx_b[c])],
                axis=2,
            )
        if ni:
            m["xi_img"] = _gather_tiles(img_bf, idx_i[c])
        if nt:
            m["xt_txt"] = _gather_tiles(txt_bf, idx_t[c])
        in_maps.append(m)

    trace = bool(int(os.environ.get("KERNEL_PROFILE", "0")))
    if trace:
        results, exec_ns, prof_json = _profiled_run(nc, in_maps)
        LAST_EXEC_NS = exec_ns
        LAST_PROFILE = prof_json

        class _R:
            pass

        res = _R()
        res.results = results
    else:
        res = run_bass_kernel_spmd(nc, in_maps, core_ids=list(range(NCORES)))
        LAST_EXEC_NS = None
        LAST_PROFILE = None

    logits = np.empty((img.shape[0], NCLS), dtype=np.float32)
    for c in range(NCORES):
        r = res.results[c]
        for name, idx in (("outb", idx_b[c]), ("outi", idx_i[c]), ("outt", idx_t[c])):
            if name in r:
                valid = idx >= 0
                flat = np.ascontiguousarray(r[name].transpose(0, 2, 1)).reshape(
                    -1, NCLS
                )
                logits[idx[valid]] = flat[valid]

    if none.any():
        # reference: fused = 0 -> logits = relu(bc1) @ Wc2 + bc2 (constant)
        row = (
            np.maximum(inputs["bc1"].astype(np.float32), 0.0) @ inputs["Wc2"]
            + inputs["bc2"]
        )
        logits[none] = row
    return logits


# revision 23
# speedup vs baseline: 1.5166x; 1.0093x over previous
"""AECF multimodal fusion kernel for 8 TRN2 NeuronCores.

Strategy:
  - Host-side routing (part of sharding): rows are sorted into three branch
    groups (both modalities present / only-image / only-text) using the same
    norm>1e-6 predicate as the reference. Each group is dealt evenly across
    the 8 cores and padded to a tile multiple; the NEFF is compiled with the
    actual per-core group sizes (compile happens inside kernel(), after the
    inputs are known), so the graph is static and identical on all cores.
  - Tile-major feature-major layout: inputs are shipped as [ntile, 128, 4, T]
    bf16 blocks (4KB contiguous per partition per tile), outputs leave as
    [ntile, 80, T] f32 blocks.
  - bf16 storage/compute, f32 PSUM accumulation.
  - Algebra: scores only enter through a 2-way softmax, so
    attn_img = sigmoid(s_img - s_txt) with s = enc @ Wk_eff,
    Wk_eff[:, h] = Wk[:, 64h:64h+64] @ q[h] / 8 (bk cancels in the diff).
    There is no nonlinearity between the post-attention projection and the
    classifier's first layer, so Wo, W_fp, Wc1 merge into Wofc1 [256,256]:
      h1pre = pooled @ Wofc1 + bh1
            = enc_t @ (Wv@Wofc1) + (A * v_d) @ Wofc1 + bh1
    with v_d = (enc_i - enc_t) @ Wv and A = head-broadcast(attn) (PE matmul
    with a 0/1 expander).  v_txt is never materialized.  Only-branches use
    Wipc1 = W_ip@Wc1 / Wtpc1 = W_tp@Wc1.
  - Depth-3 software pipeline per both-tile, ordered so the PE queue (the
    bottleneck engine) never waits on ACT/DVE round trips; PSUM tiles span
    2 banks so evacuations/relus are single ops over [128, 1024].
"""

import os
import sys

if "/opt/trn_rl_repo" not in sys.path:
    sys.path.insert(0, "/opt/trn_rl_repo")

import numpy as np
import ml_dtypes

import concourse.bass as bass
import concourse.bacc as bacc
import concourse.tile as tile
from concourse import mybir
from concourse.bass_utils import run_bass_kernel_spmd

BF = mybir.dt.bfloat16
F32 = mybir.dt.float32
AF = mybir.ActivationFunctionType
OP = mybir.AluOpType

H = 256
ID = 512
TD = 512
NCLS = 80
NH = 4
HD = 64
B = 131072
NCORES = 8
T = 512  # batch-tile (free-dim) size; one psum bank of f32 per 128-chunk

LAST_EXEC_NS = None
LAST_PROFILE = None

_GRAPH_CACHE = {}


def _ntl(n):
    """Number of tiles and last-tile length for a group of n columns."""
    ntiles = (n + T - 1) // T
    tl_last = n - (ntiles - 1) * T if ntiles else 0
    return ntiles, tl_last


def _build_graph(nb, ni, nt, zero_bias):
    """Build the SPMD graph for per-core group column counts nb/ni/nt
    (each a multiple of 16, possibly 0). Inputs/outputs are flat
    tile-major blocks so the final tile of each group can be short."""
    nc = bacc.Bacc()
    nbt, tlb = _ntl(nb)
    nit, tli = _ntl(ni)
    ntt, tlt = _ntl(nt)

    # ---- DRAM I/O ----
    dram = {}
    if nbt:
        dram["xb"] = nc.dram_tensor("xb", [128, 8 * nb], BF, kind="ExternalInput")
        dram["outb"] = nc.dram_tensor("outb", [NCLS, nb], F32, kind="ExternalOutput")
    if nit:
        dram["xi_img"] = nc.dram_tensor("xi_img", [128, 4 * ni], BF, kind="ExternalInput")
        dram["outi"] = nc.dram_tensor("outi", [NCLS, ni], F32, kind="ExternalOutput")
    if ntt:
        dram["xt_txt"] = nc.dram_tensor("xt_txt", [128, 4 * nt], BF, kind="ExternalInput")
        dram["outt"] = nc.dram_tensor("outt", [NCLS, nt], F32, kind="ExternalOutput")

    wspec = {
        "wie": ([128, 4, H], BF),
        "wte": ([128, 4, H], BF),
        "wkeff": ([128, 2, NH], BF),
        "emat": ([NH, 2, 128], BF),
        "wv": ([128, 2, H], BF),
        "wvofc1": ([128, 2, H], BF),
        "wofc1": ([128, 2, H], BF),
        "wipc1": ([128, 2, H], BF),
        "wtpc1": ([128, 2, H], BF),
        "wc2": ([128, 2, NCLS], BF),
        "bie": ([128, 2], F32),
        "bte": ([128, 2], F32),
        "bh1b": ([128, 2], F32),
        "bh1i": ([128, 2], F32),
        "bh1t": ([128, 2], F32),
        "bc2": ([128, 1], F32),
    }
    for name, (shape, dt) in wspec.items():
        dram[name] = nc.dram_tensor(name, shape, dt, kind="ExternalInput")

    with tile.TileContext(nc) as tc:
        with (
            tc.tile_pool(name="wpool", bufs=1) as wpool,
            tc.tile_pool(name="work", bufs=2) as wp,
            tc.tile_pool(name="psum", bufs=1, space="PSUM") as pp,
        ):
            w = {}
            for name, (shape, dt) in wspec.items():
                w[name] = wpool.tile(shape, dt, tag=name, name=name)
                # encoder weights ride the sync queue ahead of the input
                # tiles; everything else streams on gpsimd
                eng = nc.sync if name in ("wie", "wte") else nc.gpsimd
                eng.dma_start(w[name][:], dram[name][:])

            def relu_evac(dst, ps, tag2, tl):
                """psum [128,2,:tl] -> sbuf bf16 with relu (+bias per m-half)."""
                if zero_bias:
                    nc.scalar.activation(dst[:, :, :tl], ps[:, :, :tl], AF.Relu)
                else:
                    for m in range(2):
                        nc.scalar.activation(
                            dst[:, m, :tl], ps[:, m, :tl], AF.Relu,
                            bias=w[tag2][:, m : m + 1],
                        )

            # ================= both-modality pipeline =================
            # Stages per tile i:
            #   S1: input DMAs      S2: encoders (PE 16) + relus + denc
            #   S3a: wkeff (PE 2) + sigmoid     S3b: vd (PE 4) + emat (PE 2)
            #        + vd evac + tmp = A*vd
            #   S4: h1pre (PE 8: enc_t@Wvofc1 + tmp@Wofc1) + h1 relu
            #   S5: wc2 (PE 2) + out evac + out DMA
            # Iteration i emits: S3a(i), S2(i+1), S3b(i), S4(i-1), S5(i-2),
            # S1(i+2) so the in-order PE queue never waits on ACT/DVE.
            X, ENC, DNC, ATT, TMP, H1 = {}, {}, {}, {}, {}, {}

            def b_tl(i):
                return T if i < nbt - 1 else tlb

            def b_s1(i):
                tl = b_tl(i)
                off = 8 * T * i
                xb = wp.tile([128, 8, T], BF, tag="xi", bufs=4, name="xb")
                nc.sync.dma_start(
                    xb[:, 0:4, :tl],
                    dram["xb"][:, off : off + 4 * tl].rearrange(
                        "p (k c) -> p k c", k=4),
                )
                nc.scalar.dma_start(
                    xb[:, 4:8, :tl],
                    dram["xb"][:, off + 4 * tl : off + 8 * tl].rearrange(
                        "p (k c) -> p k c", k=4),
                )
                X[i] = xb

            def b_s2(i):
                tl = b_tl(i)
                xb = X.pop(i)
                pei = pp.tile([128, 2, T], F32, tag="ps_big", bufs=3, name="pei")
                for m in range(2):
                    for k in range(4):
                        nc.tensor.matmul(
                            pei[:, m, :tl], w["wie"][:, k, m * 128 : (m + 1) * 128],
                            xb[:, k, :tl], start=(k == 0), stop=(k == 3),
                        )
                enci = wp.tile([128, 2, T], BF, tag="enci", bufs=2, name="enci")
                relu_evac(enci, pei, "bie", tl)
                pet = pp.tile([128, 2, T], F32, tag="ps_big", bufs=3, name="pet")
                for m in range(2):
                    for k in range(4):
                        nc.tensor.matmul(
                            pet[:, m, :tl], w["wte"][:, k, m * 128 : (m + 1) * 128],
                            xb[:, 4 + k, :tl], start=(k == 0), stop=(k == 3),
                        )
                enct = wp.tile([128, 2, T], BF, tag="enct", bufs=3, name="enct")
                relu_evac(enct, pet, "bte", tl)
                denc = wp.tile([128, 2, T], BF, tag="denc", bufs=3, name="denc")
                nc.vector.tensor_tensor(denc[:, :, :tl], enci[:, :, :tl],
                                        enct[:, :, :tl], op=OP.subtract)
                ENC[i] = enct
                DNC[i] = denc

            def b_s3a(i):
                tl = b_tl(i)
                d = DNC[i]
                pss = pp.tile([128, T], F32, tag="ps_small", bufs=2, name="pss")
                for k in range(2):
                    nc.tensor.matmul(pss[:NH, :tl], w["wkeff"][:, k, :],
                                     d[:, k, :tl], start=(k == 0), stop=(k == 1))
                att = wp.tile([NH, T], BF, tag="att", bufs=3, name="att")
                nc.scalar.activation(att[:, :tl], pss[:NH, :tl], AF.Sigmoid)
                ATT[i] = att

            def b_s3b(i):
                tl = b_tl(i)
                d = DNC.pop(i)
                att = ATT.pop(i)
                psv = pp.tile([128, 2, T], F32, tag="ps_big", bufs=3, name="psv")
                for m in range(2):
                    for k in range(2):
                        nc.tensor.matmul(
                            psv[:, m, :tl], w["wv"][:, k, m * 128 : (m + 1) * 128],
                            d[:, k, :tl], start=(k == 0), stop=(k == 1),
                        )
                psa = pp.tile([128, 2, T], F32, tag="ps_big", bufs=3, name="psa")
                for m in range(2):
                    nc.tensor.matmul(psa[:, m, :tl], w["emat"][:, m, :],
                                     att[:, :tl], start=True, stop=True)
                vd = wp.tile([128, 2, T], BF, tag="vd", bufs=2, name="vd")
                nc.vector.tensor_copy(vd[:, :, :tl], psv[:, :, :tl])
                tmp = wp.tile([128, 2, T], BF, tag="tmp", bufs=3, name="tmp")
                nc.vector.tensor_tensor(tmp[:, :, :tl], psa[:, :, :tl],
                                        vd[:, :, :tl], op=OP.mult)
                TMP[i] = tmp

            def b_s4(i):
                tl = b_tl(i)
                enct = ENC.pop(i)
                tmp = TMP.pop(i)
                psh = pp.tile([128, 2, T], F32, tag="ps_big", bufs=3, name="psh")
                for m in range(2):
                    ms = slice(m * 128, (m + 1) * 128)
                    nc.tensor.matmul(psh[:, m, :tl], w["wvofc1"][:, 0, ms],
                                     enct[:, 0, :tl], start=True, stop=False)
                    nc.tensor.matmul(psh[:, m, :tl], w["wvofc1"][:, 1, ms],
                                     enct[:, 1, :tl], start=False, stop=False)
                    nc.tensor.matmul(psh[:, m, :tl], w["wofc1"][:, 0, ms],
                                     tmp[:, 0, :tl], start=False, stop=False)
                    nc.tensor.matmul(psh[:, m, :tl], w["wofc1"][:, 1, ms],
                                     tmp[:, 1, :tl], start=False, stop=True)
                h1 = wp.tile([128, 2, T], BF, tag="h1", bufs=3, name="h1")
                relu_evac(h1, psh, "bh1b", tl)
                H1[i] = h1

            def b_s5(i):
                tl = b_tl(i)
                h1 = H1.pop(i)
                pso = pp.tile([128, T], F32, tag="ps_small", bufs=2, name="pso")
                for k in range(2):
                    nc.tensor.matmul(pso[:NCLS, :tl], w["wc2"][:, k, :],
                                     h1[:, k, :tl], start=(k == 0), stop=(k == 1))
                osb = wp.tile([NCLS, T], F32, tag="osb", bufs=3, name="osb")
                if zero_bias:
                    nc.vector.tensor_copy(osb[:, :tl], pso[:NCLS, :tl])
                else:
                    nc.vector.tensor_scalar_add(osb[:, :tl], pso[:NCLS, :tl],
                                                w["bc2"][:NCLS, :])
                nc.gpsimd.dma_start(dram["outb"][:, T * i : T * i + tl],
                                    osb[:, :tl])

            if nbt:
                for i0 in range(min(3, nbt)):
                    b_s1(i0)
                b_s2(0)
                for i in range(nbt + 2):
                    if i < nbt:
                        b_s3a(i)
                    if i + 1 < nbt:
                        b_s2(i + 1)
                    if i < nbt:
                        b_s3b(i)
                    if 0 <= i - 1 < nbt:
                        b_s4(i - 1)
                    if 0 <= i - 2 < nbt:
                        b_s5(i - 2)
                    if i + 3 < nbt:
                        b_s1(i + 3)

            # ================= single-modality pipelines =================
            # out = relu(enc @ Wpc1 + bh1) @ Wc2 + bc2, enc = relu(x@We+be)
            def only_pipe(x_dram, out_dram, ntiles, tl_last, wenc, benc, wproj,
                          bproj, out_eng):
                XO, ENO, HO = {}, {}, {}

                def o_tl(j):
                    return T if j < ntiles - 1 else tl_last

                def o_s1(j):
                    tl = o_tl(j)
                    off = 4 * T * j
                    xo = wp.tile([128, 4, T], BF, tag="xi", bufs=4, name="xo")
                    nc.sync.dma_start(
                        xo[:, :, :tl],
                        x_dram[:, off : off + 4 * tl].rearrange(
                            "p (k c) -> p k c", k=4),
                    )
                    XO[j] = xo

                def o_s2(j):
                    tl = o_tl(j)
                    xo = XO.pop(j)
                    pe = pp.tile([128, 2, T], F32, tag="ps_big", bufs=3, name="peo")
                    for m in range(2):
                        for k in range(4):
                            nc.tensor.matmul(
                                pe[:, m, :tl], w[wenc][:, k, m * 128 : (m + 1) * 128],
                                xo[:, k, :tl], start=(k == 0), stop=(k == 3),
                            )
                    eno = wp.tile([128, 2, T], BF, tag="enct", bufs=3, name="eno")
                    relu_evac(eno, pe, benc, tl)
                    ENO[j] = eno

                def o_s3(j):
                    tl = o_tl(j)
                    eno = ENO.pop(j)
                    psh = pp.tile([128, 2, T], F32, tag="ps_big", bufs=3, name="psho")
                    for m in range(2):
                        ms = slice(m * 128, (m + 1) * 128)
                        for k in range(2):
                            nc.tensor.matmul(psh[:, m, :tl], w[wproj][:, k, ms],
                                             eno[:, k, :tl], start=(k == 0),
                                             stop=(k == 1))
                    h1 = wp.tile([128, 2, T], BF, tag="h1", bufs=3, name="h1o")
                    if zero_bias:
                        nc.vector.tensor_scalar_max(h1[:, :, :tl], psh[:, :, :tl],
                                                    0.0)
                    else:
                        for m in range(2):
                            nc.scalar.activation(
                                h1[:, m, :tl], psh[:, m, :tl], AF.Relu,
                                bias=w[bproj][:, m : m + 1],
                            )
                    HO[j] = h1

                def o_s4(j):
                    tl = o_tl(j)
                    h1 = HO.pop(j)
                    pso = pp.tile([128, T], F32, tag="ps_small", bufs=2, name="psoo")
                    for k in range(2):
                        nc.tensor.matmul(pso[:NCLS, :tl], w["wc2"][:, k, :],
                                         h1[:, k, :tl], start=(k == 0),
                                         stop=(k == 1))
                    osb = wp.tile([NCLS, T], F32, tag="osb", bufs=3, name="osbo")
                    if zero_bias:
                        nc.vector.tensor_copy(osb[:, :tl], pso[:NCLS, :tl])
                    else:
                        nc.vector.tensor_scalar_add(osb[:, :tl], pso[:NCLS, :tl],
                                                    w["bc2"][:NCLS, :])
                    out_eng.dma_start(out_dram[:, T * j : T * j + tl], osb[:, :tl])

                for j0 in range(min(3, ntiles)):
                    o_s1(j0)
                o_s2(0)
                for j in range(ntiles + 1):
                    if j + 1 < ntiles:
                        o_s2(j + 1)
                    if j < ntiles:
                        o_s3(j)
                    if 0 <= j - 1 < ntiles:
                        o_s4(j - 1)
                    if j + 3 < ntiles:
                        o_s1(j + 3)

            if nit:
                only_pipe(dram["xi_img"], dram["outi"], nit, tli, "wie", "bie",
                          "wipc1", "bh1i", nc.gpsimd)
            if ntt:
                only_pipe(dram["xt_txt"], dram["outt"], ntt, tlt, "wte", "bte",
                          "wtpc1", "bh1t", nc.scalar)

    nc.compile()
    return nc


def _prep_weights(inp):
    """Host-side weight prep: fold/merge/transpose into the device layouts."""
    f32 = np.float32
    q = (inp["fusion_query"].reshape(1, H).astype(f32) @ inp["Wq"] + inp["bq"]).reshape(
        NH, HD
    )
    wkeff = np.zeros((H, NH), f32)
    for h in range(NH):
        wkeff[:, h] = inp["Wk"][:, h * HD : (h + 1) * HD] @ q[h] / np.sqrt(HD)
    wof = inp["Wo"].astype(f32) @ inp["W_fp"]
    bof = inp["bo"].astype(f32) @ inp["W_fp"] + inp["b_fp"]
    wofc1 = wof @ inp["Wc1"]
    wvofc1 = inp["Wv"].astype(f32) @ wofc1
    bh1b = inp["bv"].astype(f32) @ wofc1 + bof @ inp["Wc1"] + inp["bc1"]
    wipc1 = inp["W_ip"].astype(f32) @ inp["Wc1"]
    bh1i = inp["b_ip"].astype(f32) @ inp["Wc1"] + inp["bc1"]
    wtpc1 = inp["W_tp"].astype(f32) @ inp["Wc1"]
    bh1t = inp["b_tp"].astype(f32) @ inp["Wc1"] + inp["bc1"]
    emat = np.zeros((NH, H), f32)
    for h in range(NH):
        emat[h, h * HD : (h + 1) * HD] = 1.0

    def ktile(a, kt):  # [K, M] -> [128, kt, M]
        return np.ascontiguousarray(
            a.reshape(kt, 128, a.shape[1]).transpose(1, 0, 2)
        )

    bf = ml_dtypes.bfloat16
    out = {
        "wie": ktile(inp["W_ie"], 4).astype(bf),
        "wte": ktile(inp["W_te"], 4).astype(bf),
        "wkeff": ktile(wkeff, 2).astype(bf),
        "emat": np.ascontiguousarray(emat.reshape(NH, 2, 128)).astype(bf),
        "wv": ktile(inp["Wv"].astype(f32), 2).astype(bf),
        "wvofc1": ktile(wvofc1, 2).astype(bf),
        "wofc1": ktile(wofc1, 2).astype(bf),
        "wipc1": ktile(wipc1, 2).astype(bf),
        "wtpc1": ktile(wtpc1, 2).astype(bf),
        "wc2": ktile(inp["Wc2"].astype(f32), 2).astype(bf),
        "bie": np.ascontiguousarray(inp["b_ie"].reshape(2, 128).T).astype(f32),
        "bte": np.ascontiguousarray(inp["b_te"].reshape(2, 128).T).astype(f32),
        "bh1b": np.ascontiguousarray(bh1b.reshape(2, 128).T).astype(f32),
        "bh1i": np.ascontiguousarray(bh1i.reshape(2, 128).T).astype(f32),
        "bh1t": np.ascontiguousarray(bh1t.reshape(2, 128).T).astype(f32),
        "bc2": np.ascontiguousarray(
            np.pad(inp["bc2"].astype(f32), (0, 128 - NCLS)).reshape(128, 1)
        ),
    }
    return out


def _split_pad(idx):
    """Split index array across cores evenly; pad each core's slice to a
    multiple of 16 with -1. Returns list of per-core padded index arrays
    (all the same length)."""
    per = [idx[c::NCORES] for c in range(NCORES)]
    n = max(len(p) for p in per)
    npad = ((n + 15) // 16) * 16 if n else 0
    out = []
    for p in per:
        a = np.full(npad, -1, dtype=np.int64)
        a[: len(p)] = p
        out.append(a)
    return out


def _tile_blocks(x_bf, idx):
    """Rows idx of x (with -1 -> zero row) as a list of feature-major
    tile blocks [128, 4, tl]: block[j][p, k, c] = x[idx[j*T+c], k*128+p]."""
    n = len(idx)
    g = np.zeros((n, x_bf.shape[1]), dtype=x_bf.dtype)
    valid = idx >= 0
    g[valid] = x_bf[idx[valid]]
    ntiles, tl_last = _ntl(n)
    blocks = []
    for j in range(ntiles):
        tl = T if j < ntiles - 1 else tl_last
        blk = g[j * T : j * T + tl]
        blocks.append(np.ascontiguousarray(blk.reshape(tl, 4, 128).transpose(2, 1, 0)))
    return blocks


def _flat_single(x_bf, idx):
    """[128, 4*n] flat tile-major input blocks."""
    return np.concatenate(
        [b.reshape(128, -1) for b in _tile_blocks(x_bf, idx)], axis=1
    )


def _flat_pair(img_bf, txt_bf, idx):
    """[128, 8*n]: per tile, img block cols then txt block cols."""
    bi = _tile_blocks(img_bf, idx)
    bt = _tile_blocks(txt_bf, idx)
    return np.concatenate(
        [np.concatenate([a.reshape(128, -1), b.reshape(128, -1)], axis=1)
         for a, b in zip(bi, bt)],
        axis=1,
    )


def _ntff_hook():
    """Build the (output_dir, device_ids) -> contextmanager NTFF profile
    hook directly via ctypes on the axon PJRT .so (the image's antenv lacks
    axon_hooks, so the boot-time registration was skipped)."""
    import ctypes
    import contextlib

    so_path = "/opt/axon/libaxon_pjrt.so"
    lib = ctypes.CDLL(so_path)
    if not hasattr(lib, "axon_start_nrt_profile"):
        return None
    lib.axon_start_nrt_profile.argtypes = [
        ctypes.POINTER(ctypes.c_int64),
        ctypes.c_size_t,
    ]
    lib.axon_start_nrt_profile.restype = ctypes.c_int64
    lib.axon_stop_nrt_profile.argtypes = [ctypes.c_char_p]
    lib.axon_stop_nrt_profile.restype = ctypes.c_int64

    @contextlib.contextmanager
    def _hook(output_dir, device_ids):
        import jax

        jax.devices()
        if device_ids:
            ids = (ctypes.c_int64 * len(device_ids))(*device_ids)
            rc = lib.axon_start_nrt_profile(ids, len(device_ids))
        else:
            rc = lib.axon_start_nrt_profile(None, 0)
        if rc != 0:
            raise RuntimeError(f"axon_start_nrt_profile rc={rc}")
        try:
            yield
        finally:
            n = lib.axon_stop_nrt_profile(str(output_dir).encode())
            print(f"profile: {n} file(s) written to {output_dir}", file=sys.stderr)

    return _hook


def _profiled_run(nc, in_maps):
    """Run via PJRT with NTFF profiling; parse exec_time_ns from the trace."""
    import tempfile
    import glob as _glob

    from concourse import bass2jax
    from concourse._compat import FishPath
    import gauge.profiler

    hook = _ntff_hook()
    tmpdir = tempfile.mkdtemp(prefix="aecf_prof_")
    if hook is None:
        results = bass2jax.run_bass_via_pjrt(nc, in_maps, n_cores=NCORES)
        return results, None, None
    with hook(tmpdir, [0]):
        results = bass2jax.run_bass_via_pjrt(nc, in_maps, n_cores=NCORES)
    ntffs = _glob.glob(os.path.join(tmpdir, "*_body*.ntff"))
    if not ntffs:
        print(f"no NTFFs in {tmpdir}: {sorted(os.listdir(tmpdir))}", file=sys.stderr)
        return results, None, None
    prof = gauge.profiler.Profile(
        profile_path=FishPath(tmpdir),
        kernel_dev_mode=True,
        profile_on_exit=False,
        bass_kernel=nc.m,
        offline_processing=True,
        fname="*_body*",
        metadata={},
    )
    try:
        pres = prof.to_perfetto(model_index=(0,))
        exec_ns = pres[0].exec_time_ns if pres else None
        pjson = prof.json_path(0).path if pres else None
    except Exception as e:
        print(f"profile parse failed: {e}", file=sys.stderr)
        return results, None, None
    return results, exec_ns, pjson


def kernel(**inputs):
    global LAST_EXEC_NS, LAST_PROFILE
    img = np.asarray(inputs["image_features"], dtype=np.float32)
    txt = np.asarray(inputs["text_features"], dtype=np.float32)

    pres_i = np.linalg.norm(img, axis=1) > 1e-6
    pres_t = np.linalg.norm(txt, axis=1) > 1e-6
    both = pres_i & pres_t
    oi = pres_i & ~pres_t
    ot = ~pres_i & pres_t
    none = ~pres_i & ~pres_t

    idx_b = _split_pad(np.nonzero(both)[0])
    idx_i = _split_pad(np.nonzero(oi)[0])
    idx_t = _split_pad(np.nonzero(ot)[0])
    nb, ni, nt = len(idx_b[0]), len(idx_i[0]), len(idx_t[0])

    bias_names = ("b_ie", "b_te", "bv", "bo", "b_fp", "b_ip", "b_tp", "bc1", "bc2")
    zero_bias = all(not np.any(np.asarray(inputs[n])) for n in bias_names)
    key = (nb, ni, nt, zero_bias)
    if key not in _GRAPH_CACHE:
        _GRAPH_CACHE[key] = _build_graph(nb, ni, nt, zero_bias)
    nc = _GRAPH_CACHE[key]

    wmap = _prep_weights(inputs)
    bf = ml_dtypes.bfloat16
    img_bf = img.astype(bf)
    txt_bf = txt.astype(bf)

    in_maps = []
    for c in range(NCORES):
        m = dict(wmap)
        if nb:
            m["xb"] = _flat_pair(img_bf, txt_bf, idx_b[c])
        if ni:
            m["xi_img"] = _flat_single(img_bf, idx_i[c])
        if nt:
            m["xt_txt"] = _flat_single(txt_bf, idx_t[c])
        in_maps.append(m)

    trace = bool(int(os.environ.get("KERNEL_PROFILE", "0")))
    if trace:
        results, exec_ns, prof_json = _profiled_run(nc, in_maps)
        LAST_EXEC_NS = exec_ns
        LAST_PROFILE = prof_json

        class _R:
            pass

        res = _R()
        res.results = results
    else:
        res = run_bass_kernel_spmd(nc, in_maps, core_ids=list(range(NCORES)))
        LAST_EXEC_NS = None
        LAST_PROFILE = None

    logits = np.empty((img.shape[0], NCLS), dtype=np.float32)
    for c in range(NCORES):
        r = res.results[c]
        for name, idx in (("outb", idx_b[c]), ("outi", idx_i[c]), ("outt", idx_t[c])):
            if name in r:
                valid = idx >= 0
                logits[idx[valid]] = r[name].T[valid]

    if none.any():
        # reference: fused = 0 -> logits = relu(bc1) @ Wc2 + bc2 (constant)
        row = (
            np.maximum(inputs["bc1"].astype(np.float32), 0.0) @ inputs["Wc2"]
            + inputs["bc2"]
        )
        logits[none] = row
    return logits
